# revision 1
# baseline (speedup 1.0000x reference)
"""Multi-head attention Trainium2 kernel (8 NeuronCores, SPMD).

Problem: B=4, T=2048, n_feat=512, H=8 heads, d_k=64.
Sharding: core c -> batch b = c//2, head-half hh = c%2 (4 heads = 256 attn dims).
Each core computes, for its (b, head-half):
    Q^T/K^T projections in [o, t] layout, V in [t, o] layout (+ ones column),
    flash-style attention with scores transposed (S^T[j, i]) so the softmax
    denominator comes out of the PV matmul for free, then the partial output
    projection out^T = Wo_blk @ x^T in [o2, t] layout.
Host sums the two head-half partials per batch, transposes, adds bo.

Matmuls run in float32r (TF32-like, ~1e-4 rel err, full PE rate); exp on ACT.
"""
import sys

sys.path.insert(0, "/opt/trn_rl_repo")

import numpy as np

import concourse.bass as bass
import concourse.tile as tile
from concourse import bacc, mybir
from concourse.bass_utils import run_bass_kernel_spmd

P = 128
T = 2048
F = 512            # n_feat (projection contraction dim)
OB = 256           # per-core attention dims (4 heads x 64)
NH = 4             # local heads
DK = 64
NT = T // P        # 16 row tiles
FO = F // P        # 4 feature tiles
NSUP = 2           # i-supers per head
ISUP = T // NSUP   # 1024
NC_ = ISUP // P    # 8 chunks per super
JT = NT            # 16 j tiles
NEG = -1.0e30
EPS = 1e-8

f32 = mybir.dt.float32
f32r = mybir.dt.float32r

_CACHE = {}


def _build():
    nc = bacc.Bacc("TRN2", target_bir_lowering=False, debug=False, num_devices=8)

    xq = nc.dram_tensor("xq", (T, F), f32, kind="ExternalInput").ap()
    xk = nc.dram_tensor("xk", (T, F), f32, kind="ExternalInput").ap()
    xv = nc.dram_tensor("xv", (T, F), f32, kind="ExternalInput").ap()
    wq = nc.dram_tensor("wq", (OB, F), f32, kind="ExternalInput").ap()
    wk = nc.dram_tensor("wk", (OB, F), f32, kind="ExternalInput").ap()
    wv = nc.dram_tensor("wv", (OB, F), f32, kind="ExternalInput").ap()
    wo = nc.dram_tensor("wo", (F, OB), f32, kind="ExternalInput").ap()
    bqr = nc.dram_tensor("bqr", (P, OB // P), f32, kind="ExternalInput").ap()
    bkr = nc.dram_tensor("bkr", (P, OB // P), f32, kind="ExternalInput").ap()
    bvb = nc.dram_tensor("bvb", (P, OB), f32, kind="ExternalInput").ap()
    mb = nc.dram_tensor("mb", (P, JT), f32, kind="ExternalInput").ap()
    ident = nc.dram_tensor("ident", (P, P), f32, kind="ExternalInput").ap()
    outT = nc.dram_tensor("outT", (F, T), f32, kind="ExternalOutput").ap()

    with tile.TileContext(nc) as tc:
        with tc.tile_pool(name="const", bufs=1) as cpool, \
             tc.tile_pool(name="persist", bufs=1) as ppool, \
             tc.tile_pool(name="win", bufs=2) as wpool, \
             tc.tile_pool(name="inp", bufs=1) as ipool, \
             tc.tile_pool(name="stage", bufs=4) as spool, \
             tc.tile_pool(name="et", bufs=3) as epool, \
             tc.tile_pool(name="norm", bufs=2) as npool, \
             tc.tile_pool(name="ps", bufs=2, space="PSUM") as ps:

            def big_ps(name):
                # "big" tag: 2-bank slots shared by S^T / proj / dance tiles
                return ps.tile([P, ISUP], f32, tag="big", name=name)

            def xp_ps(name):
                # "xp" tag: 2-bank slots shared by PV accum / transpose staging
                return ps.tile([P, ISUP], f32, tag="xp", name=name)

            # ---- constants ----
            id_sb = cpool.tile([P, P], f32, tag="ident")
            nc.sync.dma_start(out=id_sb[:], in_=ident[:])
            bq_sb = cpool.tile([P, OB // P], f32, tag="bq")
            nc.gpsimd.dma_start(out=bq_sb[:], in_=bqr[:])
            bk_sb = cpool.tile([P, OB // P], f32, tag="bk")
            nc.gpsimd.dma_start(out=bk_sb[:], in_=bkr[:])
            bv_sb = cpool.tile([P, OB], f32, tag="bv")
            nc.gpsimd.dma_start(out=bv_sb[:], in_=bvb[:])
            mb_sb = cpool.tile([P, JT], f32, tag="mb")
            nc.gpsimd.dma_start(out=mb_sb[:], in_=mb[:])

            # ---- weight transpose helpers (emitted per-tensor below) ----
            wT = {}

            def emit_wT(name, wdram):
                w_sb = wpool.tile([P, OB // P, F], f32, tag="wstage")
                nc.gpsimd.dma_start(
                    out=w_sb[:], in_=wdram.rearrange("(po p) f -> p po f", p=P)
                )
                wt = cpool.tile([P, FO, OB], f32r, tag=f"w{name}T")
                for fo in range(FO):
                    tp = xp_ps(f"wtr_{name}_{fo}")
                    for po in range(OB // P):
                        nc.tensor.transpose(
                            tp[:, po * P:(po + 1) * P],
                            w_sb[:, po, fo * P:(fo + 1) * P],
                            id_sb[:],
                        )
                    nc.scalar.copy(wt[:, fo, :], tp[:, :OB])
                wT[name] = wt

            def emit_woT():
                wo_sb = wpool.tile([P, FO, OB], f32, tag="wstage")
                nc.sync.dma_start(
                    out=wo_sb[:], in_=wo.rearrange("(a p) o -> p a o", p=P)
                )
                woT = cpool.tile([DK, NH, F], f32r, tag="woT")
                for h in range(NH):
                    tp = xp_ps(f"wotr_{h}")
                    for a in range(FO):
                        nc.tensor.transpose(
                            tp[:DK, a * P:(a + 1) * P],
                            wo_sb[:, a, h * DK:(h + 1) * DK],
                            id_sb[:],
                        )
                    nc.scalar.copy(woT[:, h, :], tp[:DK, :F])
                return woT

            # ---- persistent activations ----
            QT = ppool.tile([P, OB // P, T], f32r, tag="QT")
            KT = ppool.tile([P, OB // P, T], f32r, tag="KT")
            xT = ppool.tile([DK, NH, T], f32r, tag="xT")
            V2 = ppool.tile([P, NT, NH, DK + 1], f32r, tag="V2")
            one_sb = cpool.tile([P, NT * NH], f32, tag="ones")
            nc.vector.memset(one_sb[:], 1.0)
            nc.vector.tensor_copy(
                V2[:, :, :, DK:DK + 1],
                one_sb[:].rearrange("p (t h) -> p t h ()", t=NT),
            )

            # ---- phase 1: input transpose + projections ----
            def load_transposed(xdram, name):
                """x [T, F] -> inT [P, FO, T] f32r (partition = f%128)."""
                inT = ipool.tile([P, FO, T], f32r, tag="inT")
                xr = xdram.rearrange("(t p) f -> p t f", p=P)
                for g in range(NT // 2):
                    xs = spool.tile([P, 2, F], f32, tag="xs")
                    dma_eng = nc.sync if g % 2 == 0 else nc.gpsimd
                    dma_eng.dma_start(out=xs[:], in_=xr[:, 2 * g:2 * (g + 1), :])
                    t = 2 * g
                    tp = xp_ps(f"itr_{name}_{t}")
                    for i in range(2):
                        for fo in range(FO):
                            nc.tensor.transpose(
                                tp[:, i * F + fo * P:i * F + (fo + 1) * P],
                                xs[:, i, fo * P:(fo + 1) * P],
                                id_sb[:],
                            )
                    src_ap = tp[:, :2 * F].rearrange(
                        "p (i fo q) -> p fo i q", i=2, fo=FO
                    )
                    dst_ap = inT[:, :, t * P:(t + 2) * P].rearrange(
                        "p fo (i q) -> p fo i q", i=2
                    )
                    if g % 2 == 0:
                        nc.scalar.copy(dst_ap, src_ap)
                    else:
                        nc.vector.tensor_copy(dst_ap, src_ap)
                return inT

            def emit_qk_proj(name, bias_sb, dst, inT, po):
                for c in range(T // F):
                    pp = big_ps(f"proj_{name}_{po}_{c}")
                    for fo in range(FO):
                        nc.tensor.matmul(
                            pp[:, :F],
                            wT[name][:, fo, po * P:(po + 1) * P],
                            inT[:, fo, c * F:(c + 1) * F],
                            start=(fo == 0),
                            stop=(fo == FO - 1),
                        )
                    nc.vector.tensor_scalar_add(
                        dst[:, po, c * F:(c + 1) * F],
                        pp[:, :F],
                        bias_sb[:, po:po + 1],
                    )

            # K first (scores need all of KT po=0), then Q, then V
            emit_wT("k", wk)
            inT_k = load_transposed(xk, "k")
            emit_qk_proj("k", bk_sb, KT, inT_k, 0)
            emit_qk_proj("k", bk_sb, KT, inT_k, 1)
            emit_wT("q", wq)
            inT_q = load_transposed(xq, "q")
            emit_qk_proj("q", bq_sb, QT, inT_q, 0)
            emit_qk_proj("q", bq_sb, QT, inT_q, 1)

            # V: natural [t, o] layout + bias, interleaved into V2
            emit_wT("v", wv)
            inT_v = load_transposed(xv, "v")
            for t in range(NT):
                pp = big_ps(f"proj_v_{t}")
                for fo in range(FO):
                    nc.tensor.matmul(
                        pp[:, :OB],
                        inT_v[:, fo, t * P:(t + 1) * P],
                        wT["v"][:, fo, :],
                        start=(fo == 0),
                        stop=(fo == FO - 1),
                    )
                nc.vector.tensor_add(
                    V2[:, t, :, 0:DK],
                    pp[:, :OB].rearrange("p (h d) -> p h d", h=NH),
                    bv_sb[:].rearrange("p (h d) -> p h d", h=NH),
                )
            # ---- phase 2: attention ----
            def emit_jloop(h, su, dance_cb=None, jt_cb=None):
                qoff = (h % 2) * DK
                qpo = h // 2
                isl = su * ISUP
                xp = xp_ps(f"xp_{h}_{su}")

                def scores(jt):
                    st = big_ps(f"st_{h}_{su}_{jt}")
                    for c in range(ISUP // F):
                        nc.tensor.matmul(
                            st[:, c * F:(c + 1) * F],
                            KT[qoff:qoff + DK, qpo, jt * P:(jt + 1) * P],
                            QT[qoff:qoff + DK, qpo, isl + c * F:isl + (c + 1) * F],
                            start=True,
                            stop=True,
                        )
                    return st

                st_prev = scores(0)
                for jt in range(JT):
                    et = epool.tile([P, ISUP], f32r, tag="et")
                    nc.scalar.activation(
                        et[:],
                        st_prev[:],
                        mybir.ActivationFunctionType.Exp,
                        bias=mb_sb[:, jt:jt + 1],
                        scale=0.125,
                    )
                    if jt + 1 < JT:
                        st_prev = scores(jt + 1)
                    for c in range(ISUP // F):
                        nc.tensor.matmul(
                            xp[:DK + 1, c * F:(c + 1) * F],
                            V2[:, jt, h, :],
                            et[:, c * F:(c + 1) * F],
                            start=(jt == 0),
                            stop=(jt == JT - 1),
                        )
                    if jt == 2 and dance_cb is not None:
                        dance_cb()
                    if jt_cb is not None:
                        jt_cb(jt)
                return xp

            def emit_norm(h, su, xp):
                isl = su * ISUP
                # Z row (partition DK of xp psum) -> SBUF
                zst = npool.tile([1, ISUP], f32, tag="zrow")
                nc.vector.tensor_copy(zst[:], xp[DK:DK + 1, :ISUP])
                # transpose Z chunks onto partitions: zcol [P, NC_]
                zcol = big_ps(f"zcol_{h}_{su}")
                for c in range(NC_):
                    nc.tensor.transpose(
                        zcol[:, c:c + 1],
                        zst[:, c * P:(c + 1) * P],
                        id_sb[0:1, 0:1],
                    )
                # r = 1 / (Z + eps), partition-parallel
                zeps = npool.tile([P, NC_], f32, tag="zeps")
                nc.vector.tensor_scalar_add(zeps[:], zcol[:, :NC_], EPS)
                rcol = npool.tile([P, NC_], f32, tag="rcol")
                nc.vector.reciprocal(rcol[:], zeps[:])
                # transpose back into spare columns of the same psum tile:
                # rT rows [NC_, P] at cols [P, 2P) (disjoint from zcol's cols)
                rT_ps = zcol[:NC_, P:2 * P]
                nc.tensor.transpose(rT_ps, rcol[:], id_sb[:])
                rT_sb = npool.tile([NC_, P], f32, tag="rT_sb")
                nc.vector.tensor_copy(rT_sb[:], rT_ps)
                # gather rows into one [1, ISUP] SBUF row (partition shift via DMA)
                rrow = npool.tile([1, ISUP], f32, tag="zrow")
                for c in range(NC_):
                    (nc.sync if c % 2 == 0 else nc.gpsimd).dma_start(
                        out=rrow[:, c * P:(c + 1) * P],
                        in_=rT_sb[c:c + 1, :P],
                    )
                # broadcast across DK partitions (gpsimd)
                rb = npool.tile([DK, ISUP], f32, tag="rb")
                nc.gpsimd.partition_broadcast(rb[:], rrow[:])
                # x^T = x'^T * r
                nc.vector.tensor_mul(
                    xT[:, h, isl:isl + ISUP],
                    xp[0:DK, :ISUP],
                    rb[:],
                )

            woT = emit_woT()

            def emit_outproj(m2, half, psf):
                os2 = spool.tile([P, 2, F], f32, tag="os2")
                for cc in range(2):
                    c = 2 * half + cc
                    pp = psf(f"op_{m2}_{c}")
                    for h in range(NH):
                        nc.tensor.matmul(
                            pp[:, :F],
                            woT[:, h, m2 * P:(m2 + 1) * P],
                            xT[:, h, c * F:(c + 1) * F],
                            start=(h == 0),
                            stop=(h == NH - 1),
                        )
                    nc.vector.tensor_copy(os2[:, cc, :], pp[:, :F])
                nc.sync.dma_start(
                    out=outT[m2 * P:(m2 + 1) * P, half * 2 * F:(half + 1) * 2 * F],
                    in_=os2[:].rearrange("p c f -> p (c f)"),
                )

            pairs = [(h, su) for h in range(NH) for su in range(NSUP)]
            pending = [None]

            def dance_cb():
                if pending[0] is not None:
                    emit_norm(*pending[0])
                    pending[0] = None

            for idx, (h, su) in enumerate(pairs):
                if idx + 1 == len(pairs):
                    def late_cb(jt):
                        if jt == 3:
                            dance_cb()
                        elif jt in (6, 8, 10, 12):
                            emit_outproj((jt - 6) // 2, 0, xp_ps)
                    xp = emit_jloop(h, su, None, late_cb)
                else:
                    xp = emit_jloop(h, su, dance_cb)
                pending[0] = (h, su, xp)
            emit_norm(*pending[0])

            # ---- phase 3: remaining output projection (columns su=1) ----
            for m2 in range(F // P):
                emit_outproj(m2, 1, big_ps)

    nc.compile()
    return nc


def _prep_in_maps(query, key, value, mask, Wq, bq, Wk, bk, Wv, bv, Wo):
    ident = np.eye(P, dtype=np.float32)
    in_maps = []
    for c in range(8):
        b = c // 2
        hh = c % 2
        ob = slice(hh * OB, (hh + 1) * OB)
        mbias = np.where(mask[b, 0, :] == 0, np.float32(NEG), np.float32(0.0))
        mbias = np.ascontiguousarray(mbias.reshape(JT, P).T)
        in_maps.append({
            "xq": np.ascontiguousarray(query[b]),
            "xk": np.ascontiguousarray(key[b]),
            "xv": np.ascontiguousarray(value[b]),
            "wq": np.ascontiguousarray(Wq[ob, :]),
            "wk": np.ascontiguousarray(Wk[ob, :]),
            "wv": np.ascontiguousarray(Wv[ob, :]),
            "wo": np.ascontiguousarray(Wo[:, ob]),
            "bqr": np.ascontiguousarray(bq[ob].reshape(OB // P, P).T),
            "bkr": np.ascontiguousarray(bk[ob].reshape(OB // P, P).T),
            "bvb": np.ascontiguousarray(np.tile(bv[ob][None, :], (P, 1))),
            "mb": mbias,
            "ident": ident,
        })
    return in_maps


def kernel(query, key, value, mask, Wq, bq, Wk, bk, Wv, bv, Wo, bo):
    query = np.asarray(query, dtype=np.float32)
    key = np.asarray(key, dtype=np.float32)
    value = np.asarray(value, dtype=np.float32)
    mask = np.asarray(mask)
    Wq = np.asarray(Wq, dtype=np.float32)
    bq = np.asarray(bq, dtype=np.float32)
    Wk = np.asarray(Wk, dtype=np.float32)
    bk = np.asarray(bk, dtype=np.float32)
    Wv = np.asarray(Wv, dtype=np.float32)
    bv = np.asarray(bv, dtype=np.float32)
    Wo = np.asarray(Wo, dtype=np.float32)
    bo = np.asarray(bo, dtype=np.float32)

    if "nc" not in _CACHE:
        _CACHE["nc"] = _build()
    nc = _CACHE["nc"]

    B = query.shape[0]
    in_maps = _prep_in_maps(query, key, value, mask, Wq, bq, Wk, bk, Wv, bv, Wo)
    res = run_bass_kernel_spmd(nc, in_maps, core_ids=list(range(8)))

    out = np.empty((B, T, F), dtype=np.float32)
    for b in range(B):
        acc = res.results[2 * b]["outT"] + res.results[2 * b + 1]["outT"]
        out[b] = acc.T + bo[None, :]
    return out



# revision 5
# speedup vs baseline: 1.4641x; 1.4641x over previous
"""Multi-head attention Trainium2 kernel (8 NeuronCores, SPMD), v2.

Problem: B=4, T=2048, n_feat=512, H=8 heads, d_k=64.
Sharding: core c -> batch b = c//2, head-half hh = c%2 (4 heads = 256 attn dims).

Design (ACT-exp-bound, ~133us of exp on the scalar engine is the floor):
- Host pre-transposes + bf16-casts activations (x^T [512,2048]) and weights,
  so no on-device transposes for the projection phase.
- Q^T/K^T projections in [o, t] layout (o on partitions), V in [t, o] layout
  with a ones column (softmax denominator falls out of the PV matmul).
- Scores S^T[j, i] per (head, i-super of 1024): j-tile of 128 rows at a time,
  exp on ACT (scale=0.125 folded in, no bias - mask folded into V rows).
- PV reoriented: out[i-block(128), dk+1] with et as stationary operand: uses
  all 128 output partitions (half the matmul column-streams of the [dk,i]
  orientation) and leaves the denominator as a per-partition scalar, so the
  norm is a reciprocal + per-partition scalar multiply on DVE. No transpose
  dance.
- Normalized x tiles transposed on PE (bf16, cheap) into x^T for the output
  projection; out^T = Wo_blk @ x^T accumulated over 4 heads.
- Projections / output projections / norm-transposes are interleaved into the
  attention jt-loops so the ACT exp stream never starves.
Host sums the two head-half partials per batch, transposes, adds bo.
"""
import sys

sys.path.insert(0, "/opt/trn_rl_repo")

import numpy as np
import ml_dtypes

import concourse.bass as bass
import concourse.tile as tile
from concourse import bacc, mybir
from concourse.bass_utils import run_bass_kernel_spmd

P = 128
T = 2048
F = 512            # n_feat (projection contraction dim)
OB = 256           # per-core attention dims (4 heads x 64)
NH = 4             # local heads
DK = 64
NT = T // P        # 16 row tiles
FO = F // P        # 4 feature tiles
NSUP = 2           # i-supers per head
ISUP = T // NSUP   # 1024
NIB = ISUP // P    # 8 i-blocks per super
JT = NT            # 16 j tiles
EPS = 1e-8

f32 = mybir.dt.float32
bf16 = mybir.dt.bfloat16
BF = ml_dtypes.bfloat16

_CACHE = {}


def _build(mask_ones: bool):
    nc = bacc.Bacc("TRN2", target_bir_lowering=False, debug=False, num_devices=8)

    xk = nc.dram_tensor("xk", (F, T), bf16, kind="ExternalInput").ap()
    xq = nc.dram_tensor("xq", (F, T), bf16, kind="ExternalInput").ap()
    xv = nc.dram_tensor("xv", (F, T), bf16, kind="ExternalInput").ap()
    wk = nc.dram_tensor("wk", (F, OB), bf16, kind="ExternalInput").ap()
    wq = nc.dram_tensor("wq", (F, OB), bf16, kind="ExternalInput").ap()
    wv = nc.dram_tensor("wv", (F, OB), bf16, kind="ExternalInput").ap()
    wo = nc.dram_tensor("wo", (OB, F), bf16, kind="ExternalInput").ap()
    bqc = nc.dram_tensor("bqc", (P, OB // P), f32, kind="ExternalInput").ap()
    bkc = nc.dram_tensor("bkc", (P, OB // P), f32, kind="ExternalInput").ap()
    bvb = nc.dram_tensor("bvb", (P, OB), f32, kind="ExternalInput").ap()
    ident = nc.dram_tensor("ident", (P, P), bf16, kind="ExternalInput").ap()
    if not mask_ones:
        mcol = nc.dram_tensor("mcol", (P, NT), f32, kind="ExternalInput").ap()
    outT = nc.dram_tensor("outT", (F, T), f32, kind="ExternalOutput").ap()

    xk_r = xk.rearrange("(fo p) t -> p fo t", p=P)
    xq_r = xq.rearrange("(fo p) t -> p fo t", p=P)
    xv_r = xv.rearrange("(fo p) t -> p fo t", p=P)

    with tile.TileContext(nc) as tc:
        with tc.tile_pool(name="const", bufs=1) as cpool, \
             tc.tile_pool(name="act", bufs=1) as apool, \
             tc.tile_pool(name="persist", bufs=1) as ppool, \
             tc.tile_pool(name="et", bufs=26) as epool, \
             tc.tile_pool(name="norm", bufs=2) as npool, \
             tc.tile_pool(name="out", bufs=2) as opool, \
             tc.tile_pool(name="ps_st", bufs=2, space="PSUM") as ps_st, \
             tc.tile_pool(name="ps_pp", bufs=2, space="PSUM") as ps_pp, \
             tc.tile_pool(name="ps_xp", bufs=1, space="PSUM") as ps_xp:

            # ---- SBUF tiles ----
            wk_sb = cpool.tile([P, FO, OB], bf16, tag="wk")
            wq_sb = cpool.tile([P, FO, OB], bf16, tag="wq")
            wv_sb = cpool.tile([P, FO, OB], bf16, tag="wv")
            wo_sb = cpool.tile([DK, NH, F], bf16, tag="wo")
            bq_sb = cpool.tile([P, OB // P], f32, tag="bq")
            bk_sb = cpool.tile([P, OB // P], f32, tag="bk")
            bv_sb = cpool.tile([P, OB], f32, tag="bv")
            id_sb = cpool.tile([P, P], bf16, tag="ident")
            if not mask_ones:
                mc_sb = cpool.tile([P, NT], f32, tag="mcol")
                z3_sb = cpool.tile([P, NH, 1], f32, tag="z3")

            xk_sb = apool.tile([P, FO, T], bf16, tag="xk")
            xq_sb = apool.tile([P, FO, T], bf16, tag="xq")
            xv_sb = apool.tile([P, FO, T], bf16, tag="xv")

            KT = ppool.tile([P, OB // P, T], bf16, tag="KT")
            QT = ppool.tile([P, OB // P, T], bf16, tag="QT")
            V2 = ppool.tile([P, NT, NH, DK + 1], bf16, tag="V2")
            xT2 = ppool.tile([DK, NH, T], bf16, tag="xT2")

            # ---- DMA plan (single SP queue, prioritized order) ----
            def dma_x(xr, dst, c):
                nc.sync.dma_start(
                    out=dst[:, :, c * F:(c + 1) * F], in_=xr[:, :, c * F:(c + 1) * F]
                )

            nc.sync.dma_start(out=wk_sb[:], in_=wk.rearrange("(fo p) o -> p fo o", p=P))
            dma_x(xk_r, xk_sb, 0)
            nc.sync.dma_start(out=bq_sb[:], in_=bqc[:])
            nc.sync.dma_start(out=bk_sb[:], in_=bkc[:])
            nc.sync.dma_start(out=wq_sb[:], in_=wq.rearrange("(fo p) o -> p fo o", p=P))
            dma_x(xq_r, xq_sb, 0)
            dma_x(xq_r, xq_sb, 1)
            nc.sync.dma_start(out=wv_sb[:], in_=wv.rearrange("(fo p) o -> p fo o", p=P))
            nc.sync.dma_start(out=bv_sb[:], in_=bvb[:])
            dma_x(xv_r, xv_sb, 0)
            nc.sync.dma_start(out=id_sb[:], in_=ident[:])
            if not mask_ones:
                nc.sync.dma_start(out=mc_sb[:], in_=mcol[:])
            dma_x(xk_r, xk_sb, 1)
            dma_x(xv_r, xv_sb, 1)
            dma_x(xk_r, xk_sb, 2)
            dma_x(xv_r, xv_sb, 2)
            dma_x(xk_r, xk_sb, 3)
            dma_x(xv_r, xv_sb, 3)
            dma_x(xq_r, xq_sb, 2)
            dma_x(xq_r, xq_sb, 3)
            nc.sync.dma_start(out=wo_sb[:], in_=wo.rearrange("(h p) f -> p h f", p=DK))

            # V2 ones column (or mask column)
            nc.vector.memset(V2[:, :, :, DK:DK + 1], 1.0)
            if not mask_ones:
                nc.vector.memset(z3_sb[:], 0.0)

            # ---- projection emitters ----
            def qk_chunk(w_sb, b_sb, dst, x_sb, po, c):
                pp = ps_pp.tile([P, F], f32, tag="pp", name=f"qk_{po}_{c}")
                for fo in range(FO):
                    nc.tensor.matmul(
                        pp[:, :F],
                        w_sb[:, fo, po * P:(po + 1) * P],
                        x_sb[:, fo, c * F:(c + 1) * F],
                        start=(fo == 0),
                        stop=(fo == FO - 1),
                    )
                nc.vector.tensor_scalar_add(
                    dst[:, po, c * F:(c + 1) * F], pp[:, :F], b_sb[:, po:po + 1]
                )

            def v_chunk(tb):
                pp = ps_pp.tile([P, F], f32, tag="pp", name=f"v_{tb}")
                for fo in range(FO):
                    nc.tensor.matmul(
                        pp[:, :OB],
                        xv_sb[:, fo, tb * P:(tb + 1) * P],
                        wv_sb[:, fo, :],
                        start=(fo == 0),
                        stop=(fo == FO - 1),
                    )
                if mask_ones:
                    nc.vector.tensor_add(
                        V2[:, tb, :, 0:DK],
                        pp[:, :OB].rearrange("p (h d) -> p h d", h=NH),
                        bv_sb[:].rearrange("p (h d) -> p h d", h=NH),
                    )
                else:
                    vt = npool.tile([P, NH, DK], f32, tag="vt")
                    nc.vector.tensor_add(
                        vt[:],
                        pp[:, :OB].rearrange("p (h d) -> p h d", h=NH),
                        bv_sb[:].rearrange("p (h d) -> p h d", h=NH),
                    )
                    nc.vector.tensor_scalar_mul(
                        V2[:, tb, :, 0:DK], vt[:], mc_sb[:, tb:tb + 1]
                    )
                    nc.vector.tensor_scalar_add(
                        V2[:, tb, :, DK:DK + 1], z3_sb[:], mc_sb[:, tb:tb + 1]
                    )

            # ---- head start: K po0 c0, Q po0 c0-c1 ----
            qk_chunk(wk_sb, bk_sb, KT, xk_sb, 0, 0)
            qk_chunk(wq_sb, bq_sb, QT, xq_sb, 0, 0)
            qk_chunk(wq_sb, bq_sb, QT, xq_sb, 0, 1)
            v_chunk(0)
            v_chunk(1)

            # ---- norm / transpose / outproj emitters ----
            def emit_norm(pi, xpa, xpb):
                zeps = npool.tile([P, NIB], f32, tag="zeps")
                za = xpa[:].rearrange("p (ib c) -> p ib c", c=DK + 1)[
                    :, :, DK:DK + 1].rearrange("p ib one -> p (ib one)")
                zb = xpb[:].rearrange("p (ib c) -> p ib c", c=DK + 1)[
                    :, :, DK:DK + 1].rearrange("p ib one -> p (ib one)")
                nc.vector.tensor_scalar_add(zeps[:, 0:4], za, EPS)
                nc.vector.tensor_scalar_add(zeps[:, 4:8], zb, EPS)
                rz = npool.tile([P, NIB], f32, tag="rz")
                nc.vector.reciprocal(rz[:], zeps[:])
                xn = npool.tile([P, NIB, DK], bf16, tag="xn")
                for ib in range(NIB):
                    src = xpa if ib < 4 else xpb
                    ib2 = ib % 4
                    nc.vector.tensor_scalar_mul(
                        xn[:, ib, :],
                        src[:, ib2 * (DK + 1):ib2 * (DK + 1) + DK],
                        rz[:, ib:ib + 1],
                    )
                return xn

            def emit_tp(h, su, xn):
                tp = ps_pp.tile([DK, ISUP], bf16, tag="pp", name=f"tp_{h}_{su}")
                for ib in range(NIB):
                    nc.tensor.transpose(
                        tp[:, ib * P:(ib + 1) * P], xn[:, ib, :], id_sb[:]
                    )
                nc.vector.tensor_copy(
                    xT2[:, h, su * ISUP:(su + 1) * ISUP], tp[:, :ISUP]
                )

            os2_hold = {}

            def op_group(m2, su, cc):
                if cc == 0:
                    os2_hold[m2] = opool.tile([P, 2, F], f32, tag="os2", name=f"os2_{su}_{m2}")
                os2 = os2_hold[m2]
                c = su * 2 + cc
                pp = ps_pp.tile([P, F], f32, tag="pp", name=f"op_{m2}_{c}")
                for h in range(NH):
                    nc.tensor.matmul(
                        pp[:, :F],
                        wo_sb[:, h, m2 * P:(m2 + 1) * P],
                        xT2[:, h, c * F:(c + 1) * F],
                        start=(h == 0),
                        stop=(h == NH - 1),
                    )
                nc.vector.tensor_copy(os2[:, cc, :], pp[:, :F])
                if cc == 1:
                    nc.sync.dma_start(
                        out=outT[m2 * P:(m2 + 1) * P, su * ISUP:(su + 1) * ISUP],
                        in_=os2[:].rearrange("p c f -> p (c f)"),
                    )

            # ---- per-pair interleaved extras ----
            def K_(po, c):
                return lambda: qk_chunk(wk_sb, bk_sb, KT, xk_sb, po, c)

            def Q_(po, c):
                return lambda: qk_chunk(wq_sb, bq_sb, QT, xq_sb, po, c)

            def OP_(m2, su, cc):
                return lambda: op_group(m2, su, cc)

            extras_by_pair = {
                0: {1: [K_(0, 1)], 2: [K_(0, 2)], 3: [K_(0, 3)]},
                1: {9: [K_(1, 0)], 10: [K_(1, 1)], 11: [K_(1, 2)],
                    12: [K_(1, 3)], 13: [Q_(1, 0)], 14: [Q_(1, 1)]},
                2: {8: [Q_(1, 2)], 10: [Q_(1, 3)], 12: [Q_(0, 2)],
                    14: [Q_(0, 3)]},
                5: {3: [OP_(0, 0, 0)], 5: [OP_(0, 0, 1)], 7: [OP_(1, 0, 0)],
                    9: [OP_(1, 0, 1)]},
                6: {1: [OP_(2, 0, 0)], 3: [OP_(2, 0, 1)], 5: [OP_(3, 0, 0)],
                    7: [OP_(3, 0, 1)]},
            }

            # ---- attention pairs ----
            # PV for pair p runs as 8 ib-major runs (16 consecutive
            # accumulating matmuls each -- psum regions must be accumulated in
            # one contiguous run on TRN2) interleaved into pair p+1's jt loop.
            pairs = [(su, h) for su in range(NSUP) for h in range(NH)]
            state = {}  # pair index -> dict(et=list, xpa=, xpb=, h=, su=, xn=)

            def pv_run(p, ib):
                ps = state[p]
                if ib == 0:
                    ps["xpa"] = ps_xp.tile([P, 4 * (DK + 1)], f32, tag="xpa",
                                           name=f"xpa_{p}")
                    ps["xpb"] = ps_xp.tile([P, 4 * (DK + 1)], f32, tag="xpb",
                                           name=f"xpb_{p}")
                tgt = ps["xpa"] if ib < 4 else ps["xpb"]
                ib2 = ib % 4
                hp = ps["h"]
                for jt in range(JT):
                    nc.tensor.matmul(
                        tgt[:, ib2 * (DK + 1):(ib2 + 1) * (DK + 1)],
                        ps["et"][jt][:, ib * P:(ib + 1) * P],
                        V2[:, jt, hp, :],
                        start=(jt == 0),
                        stop=(jt == JT - 1),
                    )

            def drain_pair(p):
                # norm + transposes for pair p (xp complete)
                ps = state[p]
                xn = emit_norm(p, ps["xpa"], ps["xpb"])
                emit_tp(ps["h"], ps["su"], xn)
                del state[p]

            for pi, (su, h) in enumerate(pairs):
                qoff = (h % 2) * DK
                qpo = h // 2
                isl = su * ISUP
                extras = extras_by_pair.get(pi, {})
                state[pi] = {"h": h, "su": su, "et": []}

                def scores(jt):
                    st = ps_st.tile([P, ISUP], f32, tag="st",
                                    name=f"st_{pi}_{jt}")
                    for c2 in range(ISUP // F):
                        nc.tensor.matmul(
                            st[:, c2 * F:(c2 + 1) * F],
                            KT[qoff:qoff + DK, qpo, jt * P:(jt + 1) * P],
                            QT[qoff:qoff + DK, qpo,
                               isl + c2 * F:isl + (c2 + 1) * F],
                            start=True,
                            stop=True,
                        )
                    return st

                st_prev = scores(0)
                for jt in range(JT):
                    et = epool.tile([P, ISUP], bf16, tag="et",
                                    name=f"et_{pi}_{jt}")
                    nc.scalar.activation(
                        et[:], st_prev[:], mybir.ActivationFunctionType.Exp,
                        scale=0.125,
                    )
                    state[pi]["et"].append(et)
                    if jt + 1 < JT:
                        st_prev = scores(jt + 1)
                    if pi == 0 and jt + 2 < NT:
                        v_chunk(jt + 2)
                    if pi > 0 and jt < NIB:
                        pv_run(pi - 1, jt)
                    if pi > 0 and jt == NIB:
                        drain_pair(pi - 1)
                    for fn in extras.get(jt, []):
                        fn()

            # ---- tail: last pair PV + norm + su=1 output projection ----
            last = len(pairs) - 1
            for ib in range(NIB):
                pv_run(last, ib)
            drain_pair(last)
            for m2 in range(F // P):
                op_group(m2, 1, 0)
                op_group(m2, 1, 1)

    nc.compile()
    return nc


def _prep_in_maps(query, key, value, mask, Wq, bq, Wk, bk, Wv, bv, Wo,
                  mask_ones):
    ident = np.eye(P, dtype=np.float32).astype(BF)
    B = query.shape[0]
    xTs = {}
    for b in range(B):
        m01 = (mask[b, 0, :] != 0)
        xv_full = value[b] * m01[:, None].astype(np.float32)
        xTs[b] = (
            np.ascontiguousarray(key[b].T).astype(BF),
            np.ascontiguousarray(query[b].T).astype(BF),
            np.ascontiguousarray(xv_full.T).astype(BF),
            np.ascontiguousarray(
                m01.astype(np.float32).reshape(NT, P).T) if not mask_ones
            else None,
        )
    in_maps = []
    for c in range(8):
        b = c // 2
        hh = c % 2
        ob = slice(hh * OB, (hh + 1) * OB)
        xkT, xqT, xvT, mc = xTs[b]
        m = {
            "xk": xkT,
            "xq": xqT,
            "xv": xvT,
            "wk": np.ascontiguousarray(Wk[ob, :].T).astype(BF),
            "wq": np.ascontiguousarray(Wq[ob, :].T).astype(BF),
            "wv": np.ascontiguousarray(Wv[ob, :].T).astype(BF),
            "wo": np.ascontiguousarray(Wo[:, ob].T).astype(BF),
            "bqc": np.ascontiguousarray(bq[ob].reshape(OB // P, P).T),
            "bkc": np.ascontiguousarray(bk[ob].reshape(OB // P, P).T),
            "bvb": np.ascontiguousarray(np.tile(bv[ob][None, :], (P, 1))),
            "ident": ident,
        }
        if not mask_ones:
            m["mcol"] = mc
        in_maps.append(m)
    return in_maps


def kernel(query, key, value, mask, Wq, bq, Wk, bk, Wv, bv, Wo, bo):
    query = np.asarray(query, dtype=np.float32)
    key = np.asarray(key, dtype=np.float32)
    value = np.asarray(value, dtype=np.float32)
    mask = np.asarray(mask)
    Wq = np.asarray(Wq, dtype=np.float32)
    bq = np.asarray(bq, dtype=np.float32)
    Wk = np.asarray(Wk, dtype=np.float32)
    bk = np.asarray(bk, dtype=np.float32)
    Wv = np.asarray(Wv, dtype=np.float32)
    bv = np.asarray(bv, dtype=np.float32)
    Wo = np.asarray(Wo, dtype=np.float32)
    bo = np.asarray(bo, dtype=np.float32)

    mask_ones = bool(np.all(mask != 0))
    ckey = ("nc", mask_ones)
    if ckey not in _CACHE:
        _CACHE[ckey] = _build(mask_ones)
        _CACHE["nc"] = _CACHE[ckey]  # test.py reads _CACHE["nc"]
    nc = _CACHE[ckey]

    B = query.shape[0]
    in_maps = _prep_in_maps(
        query, key, value, mask, Wq, bq, Wk, bk, Wv, bv, Wo, mask_ones
    )
    res = run_bass_kernel_spmd(nc, in_maps, core_ids=list(range(8)))

    out = np.empty((B, T, F), dtype=np.float32)
    for b in range(B):
        acc = res.results[2 * b]["outT"] + res.results[2 * b + 1]["outT"]
        out[b] = acc.T + bo[None, :]
    return out


# revision 12
# speedup vs baseline: 1.4831x; 1.0130x over previous
"""Multi-head attention Trainium2 kernel (8 NeuronCores, SPMD), v4.

Problem: B=4, T=2048, n_feat=512, H=8 heads, d_k=64.
Sharding: core c -> batch b = c//2, head-half hh = c%2 (4 heads = 256 attn dims).

Design (ACT-exp-bound: 128 exps of [128,1024] = ~133us on the scalar engine is
the floor; everything else hides under it):
- Host pre-transposes + bf16-casts activations (x^T [512,2048]) and weights.
- Q^T/K^T projections in [o, t] layout, V in [t, o] layout with a ones column
  (softmax denominator falls out of the PV matmul).
- Scores S^T[j, i] per (head, i-super of 1024); exp on ACT, bf16 out.
- PV out[i-block(128), dk+1] with et stationary. TRN2 psum accumulation must
  be one contiguous run per region (interleaved/reopened groups silently drop
  a visit), so PV for pair p runs ib-major (16 consecutive matmuls per
  region) interleaved into pair p+1's jt loop; all 16 et tiles of a pair stay
  live in SBUF.
- Norm = reciprocal + per-partition scalar mul (denominator is psum col 64).
- Normalized x transposed on PE in 64x64 quadrants (tile_position) so both
  heads of a pair-half pack onto 128 partitions -> output projection
  contracts 2x128 instead of 4x64 (half the matmuls).
- Tail: last pair's PV split into jt-halves (lo into xpa/xpb, hi into pp-tag
  psum), norm/transpose/outproj pipelined per i-half, ACT helps after the exp
  stream ends. Head: packed DMA prefix + PE warmup to beat the pstate ramp.
Host sums the two head-half partials per batch, transposes, adds bo.
"""
import sys

sys.path.insert(0, "/opt/trn_rl_repo")

import numpy as np
import ml_dtypes

import concourse.bass as bass
import concourse.tile as tile
from concourse import bacc, mybir
from concourse.bass_utils import run_bass_kernel_spmd

P = 128
T = 2048
F = 512            # n_feat (projection contraction dim)
OB = 256           # per-core attention dims (4 heads x 64)
NH = 4             # local heads
DK = 64
NT = T // P        # 16 row tiles
FO = F // P        # 4 feature tiles
NSUP = 2           # i-supers per head
ISUP = T // NSUP   # 1024
NIB = ISUP // P    # 8 i-blocks per super
JT = NT            # 16 j tiles
EPS = 1e-8
D1 = DK + 1

f32 = mybir.dt.float32
bf16 = mybir.dt.bfloat16
BF = ml_dtypes.bfloat16
Exp = mybir.ActivationFunctionType.Exp

_CACHE = {}


def _build(mask_ones: bool):
    nc = bacc.Bacc("TRN2", target_bir_lowering=False, debug=False, num_devices=8)

    xk = nc.dram_tensor("xk", (F, T), bf16, kind="ExternalInput").ap()
    xq = nc.dram_tensor("xq", (F, T), bf16, kind="ExternalInput").ap()
    xv = nc.dram_tensor("xv", (F, T), bf16, kind="ExternalInput").ap()
    wkqv = nc.dram_tensor("wkqv", (3, F, OB), bf16, kind="ExternalInput").ap()
    wo = nc.dram_tensor("wo", (OB, F), bf16, kind="ExternalInput").ap()
    bqk = nc.dram_tensor("bqk", (P, 4), f32, kind="ExternalInput").ap()
    bvb = nc.dram_tensor("bvb", (P, OB), f32, kind="ExternalInput").ap()
    ident = nc.dram_tensor("ident", (P, P), bf16, kind="ExternalInput").ap()
    if not mask_ones:
        mcol = nc.dram_tensor("mcol", (P, NT), f32, kind="ExternalInput").ap()
    outT = nc.dram_tensor("outT", (F, T), f32, kind="ExternalOutput").ap()

    xk_r = xk.rearrange("(fo p) t -> p fo t", p=P)
    xq_r = xq.rearrange("(fo p) t -> p fo t", p=P)
    xv_r = xv.rearrange("(fo p) t -> p fo t", p=P)

    with tile.TileContext(nc) as tc:
        with tc.tile_pool(name="const", bufs=1) as cpool, \
             tc.tile_pool(name="act", bufs=1) as apool, \
             tc.tile_pool(name="persist", bufs=1) as ppool, \
             tc.tile_pool(name="et", bufs=26) as epool, \
             tc.tile_pool(name="norm", bufs=2) as npool, \
             tc.tile_pool(name="out", bufs=4) as opool, \
             tc.tile_pool(name="ps_st", bufs=2, space="PSUM") as ps_st, \
             tc.tile_pool(name="ps_pp", bufs=2, space="PSUM") as ps_pp, \
             tc.tile_pool(name="ps_xp", bufs=1, space="PSUM") as ps_xp:

            # ---- SBUF tiles ----
            wkqv_sb = cpool.tile([P, 3, FO, OB], bf16, tag="wkqv")
            wo_sb = cpool.tile([P, 2, F], bf16, tag="wo")
            bqk_sb = cpool.tile([P, 4], f32, tag="bqk")
            bv_sb = cpool.tile([P, OB], f32, tag="bv")
            id_sb = cpool.tile([P, P], bf16, tag="ident")
            ws = cpool.tile([P, OB], bf16, tag="warm")
            if not mask_ones:
                mc_sb = cpool.tile([P, NT], f32, tag="mcol")
                z3_sb = cpool.tile([P, NH, 1], f32, tag="z3")

            xk_sb = apool.tile([P, FO, T], bf16, tag="xk")
            xq_sb = apool.tile([P, FO, T], bf16, tag="xq")
            xv_sb = apool.tile([P, FO, T], bf16, tag="xv")

            KT = ppool.tile([P, OB // P, T], bf16, tag="KT")
            QT = ppool.tile([P, OB // P, T], bf16, tag="QT")
            V2 = ppool.tile([P, NT, NH, D1], bf16, tag="V2")
            xT2 = ppool.tile([P, 2, T], bf16, tag="xT2")

            wk_sb = wkqv_sb[:, 0, :, :]
            wq_sb = wkqv_sb[:, 1, :, :]
            wv_sb = wkqv_sb[:, 2, :, :]
            bk_col = bqk_sb[:, 0:2]
            bq_col = bqk_sb[:, 2:4]

            # ---- PE warmup: keep PE busy through the pstate ramp ----
            nc.vector.memset(ws[:], 0.01)
            for wi in range(26):
                wp = ps_st.tile([P, OB], f32, tag="st", name=f"warm_{wi}")
                nc.tensor.matmul(
                    wp[:, :OB], ws[:, 0:P], ws[:, 0:OB], start=True, stop=True
                )

            # ---- DMA plan (single SP queue, prioritized order) ----
            def dma_x(xr, dst, c):
                nc.sync.dma_start(
                    out=dst[:, :, c * F:(c + 1) * F], in_=xr[:, :, c * F:(c + 1) * F]
                )

            nc.sync.dma_start(out=bqk_sb[:], in_=bqk[:])
            nc.sync.dma_start(
                out=wkqv_sb[:], in_=wkqv.rearrange("w (fo p) o -> p w fo o", p=P)
            )
            dma_x(xk_r, xk_sb, 0)
            dma_x(xq_r, xq_sb, 0)
            dma_x(xq_r, xq_sb, 1)
            nc.sync.dma_start(out=bv_sb[:], in_=bvb[:])
            dma_x(xv_r, xv_sb, 0)
            nc.sync.dma_start(out=id_sb[:], in_=ident[:])
            if not mask_ones:
                nc.sync.dma_start(out=mc_sb[:], in_=mcol[:])
            dma_x(xk_r, xk_sb, 1)
            dma_x(xv_r, xv_sb, 1)
            dma_x(xk_r, xk_sb, 2)
            dma_x(xv_r, xv_sb, 2)
            dma_x(xk_r, xk_sb, 3)
            dma_x(xv_r, xv_sb, 3)
            dma_x(xq_r, xq_sb, 2)
            dma_x(xq_r, xq_sb, 3)
            nc.sync.dma_start(out=wo_sb[:], in_=wo.rearrange("(oh p) f -> p oh f", p=P))

            # V2 ones column (or mask column)
            nc.vector.memset(V2[:, :, :, DK:D1], 1.0)
            if not mask_ones:
                nc.vector.memset(z3_sb[:], 0.0)

            # ---- projection emitters ----
            def qk_chunk(w_sb, b_col, dst, x_sb, po, c):
                pp = ps_pp.tile([P, F], f32, tag="pp", name=f"qk_{po}_{c}")
                for fo in range(FO):
                    nc.tensor.matmul(
                        pp[:, :F],
                        w_sb[:, fo, po * P:(po + 1) * P],
                        x_sb[:, fo, c * F:(c + 1) * F],
                        start=(fo == 0),
                        stop=(fo == FO - 1),
                    )
                nc.vector.tensor_scalar_add(
                    dst[:, po, c * F:(c + 1) * F], pp[:, :F], b_col[:, po:po + 1]
                )

            def v_chunk(tb):
                pp = ps_pp.tile([P, F], f32, tag="pp", name=f"v_{tb}")
                for fo in range(FO):
                    nc.tensor.matmul(
                        pp[:, :OB],
                        xv_sb[:, fo, tb * P:(tb + 1) * P],
                        wv_sb[:, fo, :],
                        start=(fo == 0),
                        stop=(fo == FO - 1),
                    )
                if mask_ones:
                    nc.vector.tensor_add(
                        V2[:, tb, :, 0:DK],
                        pp[:, :OB].rearrange("p (h d) -> p h d", h=NH),
                        bv_sb[:].rearrange("p (h d) -> p h d", h=NH),
                    )
                else:
                    vt = npool.tile([P, NH, DK], f32, tag="vt")
                    nc.vector.tensor_add(
                        vt[:],
                        pp[:, :OB].rearrange("p (h d) -> p h d", h=NH),
                        bv_sb[:].rearrange("p (h d) -> p h d", h=NH),
                    )
                    nc.vector.tensor_scalar_mul(
                        V2[:, tb, :, 0:DK], vt[:], mc_sb[:, tb:tb + 1]
                    )
                    nc.vector.tensor_scalar_add(
                        V2[:, tb, :, DK:D1], z3_sb[:], mc_sb[:, tb:tb + 1]
                    )

            # ---- head start: K po0 c0, Q po0 c0-c1 ----
            qk_chunk(wk_sb, bk_col, KT, xk_sb, 0, 0)
            qk_chunk(wq_sb, bq_col, QT, xq_sb, 0, 0)
            qk_chunk(wq_sb, bq_col, QT, xq_sb, 0, 1)
            v_chunk(0)
            v_chunk(1)

            # ---- norm / transpose / outproj emitters ----
            def emit_norm_mul(xn, ib, src_ap, rz, eng_act=False):
                if eng_act:
                    nc.scalar.mul(xn[:, ib, :], src_ap, rz[:, ib:ib + 1])
                else:
                    nc.vector.tensor_scalar_mul(xn[:, ib, :], src_ap,
                                                rz[:, ib:ib + 1])

            xn2_hold = {}

            def get_xn2(oh, su):
                if (oh, su) not in xn2_hold:
                    xn2_hold[(oh, su)] = npool.tile(
                        [P, NIB, 2, DK], bf16, tag="xn",
                        name=f"xn2_{oh}_{su}")
                return xn2_hold[(oh, su)]

            def emit_norm(pi, h, su, xpa, xpb):
                zeps = npool.tile([P, NIB], f32, tag="zeps")
                za = xpa[:].rearrange("p (ib c) -> p ib c", c=D1)[
                    :, :, DK:D1].rearrange("p ib one -> p (ib one)")
                zb = xpb[:].rearrange("p (ib c) -> p ib c", c=D1)[
                    :, :, DK:D1].rearrange("p ib one -> p (ib one)")
                nc.vector.tensor_scalar_add(zeps[:, 0:4], za, EPS)
                nc.vector.tensor_scalar_add(zeps[:, 4:8], zb, EPS)
                rz = npool.tile([P, NIB], f32, tag="rz")
                nc.vector.reciprocal(rz[:], zeps[:])
                xn2 = get_xn2(h // 2, su)
                for ib in range(NIB):
                    src = xpa if ib < 4 else xpb
                    ib2 = ib % 4
                    nc.vector.tensor_scalar_mul(
                        xn2[:, ib, h % 2, :],
                        src[:, ib2 * D1:ib2 * D1 + DK],
                        rz[:, ib:ib + 1],
                    )
                return xn2

            def emit_tp_ib(tp, xn2, ib):
                # full 128x128 transpose: [i, (h d)] -> [(h d), i]
                nc.tensor.transpose(
                    tp[:, ib * P:(ib + 1) * P],
                    xn2[:, ib, :, :].rearrange("p h d -> p (h d)"),
                    id_sb[:],
                )

            def emit_tp(oh, su, xn2, ib_lo=0, nib=NIB):
                tp = ps_pp.tile([P, ISUP], bf16, tag="pp",
                                name=f"tp_{oh}_{su}_{ib_lo}")
                for ib in range(ib_lo, ib_lo + nib):
                    emit_tp_ib(tp, xn2, ib)
                nc.vector.tensor_copy(
                    xT2[:, oh, su * ISUP + ib_lo * P:
                        su * ISUP + (ib_lo + nib) * P],
                    tp[:, ib_lo * P:(ib_lo + nib) * P],
                )

            os2_hold = {}

            def op_group(m2, su, cc, eng_act=False):
                if cc == 0:
                    os2_hold[m2] = opool.tile([P, 2, F], f32, tag="os2",
                                              name=f"os2_{su}_{m2}")
                os2 = os2_hold[m2]
                c = su * 2 + cc
                pp = ps_pp.tile([P, F], f32, tag="pp", name=f"op_{m2}_{c}")
                for oh in range(2):
                    nc.tensor.matmul(
                        pp[:, :F],
                        wo_sb[:, oh, m2 * P:(m2 + 1) * P],
                        xT2[:, oh, c * F:(c + 1) * F],
                        start=(oh == 0),
                        stop=(oh == 1),
                    )
                if eng_act:
                    nc.scalar.copy(os2[:, cc, :], pp[:, :F])
                else:
                    nc.vector.tensor_copy(os2[:, cc, :], pp[:, :F])
                if cc == 1:
                    nc.sync.dma_start(
                        out=outT[m2 * P:(m2 + 1) * P, su * ISUP:(su + 1) * ISUP],
                        in_=os2[:].rearrange("p c f -> p (c f)"),
                    )

            # ---- per-pair interleaved extras ----
            def K_(po, c):
                return lambda: qk_chunk(wk_sb, bk_col, KT, xk_sb, po, c)

            def Q_(po, c):
                return lambda: qk_chunk(wq_sb, bq_col, QT, xq_sb, po, c)

            def OP_(m2, su, cc):
                return lambda: op_group(m2, su, cc)

            extras_by_pair = {
                0: {1: [K_(0, 1)], 2: [K_(0, 2)], 3: [K_(0, 3)]},
                1: {9: [K_(1, 0)], 10: [K_(1, 1)], 11: [K_(1, 2)],
                    12: [K_(1, 3)], 13: [Q_(1, 0)], 14: [Q_(1, 1)]},
                2: {8: [Q_(1, 2)], 10: [Q_(1, 3)], 12: [Q_(0, 2)],
                    14: [Q_(0, 3)]},
                5: {3: [OP_(0, 0, 0)], 5: [OP_(0, 0, 1)], 7: [OP_(1, 0, 0)],
                    9: [OP_(1, 0, 1)]},
                6: {1: [OP_(2, 0, 0)], 3: [OP_(2, 0, 1)], 5: [OP_(3, 0, 0)],
                    7: [OP_(3, 0, 1)]},
            }

            # ---- attention pairs ----
            # PV for pair p: 8 ib-major runs (16 consecutive matmuls each)
            # interleaved into pair p+1's jt loop. Last pair: jt-halves.
            pairs = [(su, h) for su in range(NSUP) for h in range(NH)]
            LAST = len(pairs) - 1
            state = {}

            def pv_run(p, ib, jt0=0, njt=JT, tgt_pair=None, fresh=True):
                ps = state[p]
                if tgt_pair is None:
                    if ib == 0 and fresh:
                        ps["xpa"] = ps_xp.tile([P, 4 * D1], f32, tag="xpa",
                                               name=f"xpa_{p}")
                        ps["xpb"] = ps_xp.tile([P, 4 * D1], f32, tag="xpb",
                                               name=f"xpb_{p}")
                    tgt = ps["xpa"] if ib < 4 else ps["xpb"]
                else:
                    tgt = tgt_pair[0] if ib < 4 else tgt_pair[1]
                ib2 = ib % 4
                hp = ps["h"]
                for jj in range(njt):
                    jt = jt0 + jj
                    nc.tensor.matmul(
                        tgt[:, ib2 * D1:(ib2 + 1) * D1],
                        ps["et"][jt][:, ib * P:(ib + 1) * P],
                        V2[:, jt, hp, :],
                        start=(jj == 0),
                        stop=(jj == njt - 1),
                    )

            def drain_pair(p):
                ps = state[p]
                h, su = ps["h"], ps["su"]
                xn2 = emit_norm(p, h, su, ps["xpa"], ps["xpb"])
                if h % 2 == 1:
                    emit_tp(h // 2, su, xn2)
                    del xn2_hold[(h // 2, su)]
                del state[p]

            for pi, (su, h) in enumerate(pairs):
                qoff = (h % 2) * DK
                qpo = h // 2
                isl = su * ISUP
                extras = extras_by_pair.get(pi, {})
                state[pi] = {"h": h, "su": su, "et": []}

                def scores(jt):
                    st = ps_st.tile([P, ISUP], f32, tag="st",
                                    name=f"st_{pi}_{jt}")
                    for c2 in range(ISUP // F):
                        nc.tensor.matmul(
                            st[:, c2 * F:(c2 + 1) * F],
                            KT[qoff:qoff + DK, qpo, jt * P:(jt + 1) * P],
                            QT[qoff:qoff + DK, qpo,
                               isl + c2 * F:isl + (c2 + 1) * F],
                            start=True,
                            stop=True,
                        )
                    return st

                st_prev = scores(0)
                for jt in range(JT):
                    et = epool.tile([P, ISUP], bf16, tag="et",
                                    name=f"et_{pi}_{jt}")
                    nc.scalar.activation(et[:], st_prev[:], Exp, scale=0.125)
                    state[pi]["et"].append(et)
                    if jt + 1 < JT:
                        st_prev = scores(jt + 1)
                    if pi == 0 and jt + 2 < NT:
                        v_chunk(jt + 2)
                    if pi > 0 and jt < NIB:
                        pv_run(pi - 1, jt)
                    if pi > 0 and jt == NIB:
                        drain_pair(pi - 1)
                    if pi == LAST and jt >= NIB:
                        # lo-half runs of the last pair (jts 0-7)
                        pv_run(LAST, jt - NIB, jt0=0, njt=NIB)
                    for fn in extras.get(jt, []):
                        fn()

            # ---- tail: last pair hi-half + norm/tp/outproj per i-half ----
            ps7 = state[LAST]
            h7, su7 = ps7["h"], ps7["su"]
            # lo halves -> SBUF (overlaps the hi runs; DVE can read only one
            # PSUM operand per tensor_tensor)
            lo_sb = npool.tile([P, NIB * D1], f32, tag="losb")
            nc.vector.tensor_copy(lo_sb[:, 0:4 * D1], ps7["xpa"][:])
            nc.vector.tensor_copy(lo_sb[:, 4 * D1:NIB * D1], ps7["xpb"][:])
            hia = ps_pp.tile([P, 4 * D1], f32, tag="pp", name="hia")
            hib = ps_pp.tile([P, 4 * D1], f32, tag="pp", name="hib")
            for ib in range(NIB):
                pv_run(LAST, ib, jt0=NIB, njt=NIB, tgt_pair=(hia, hib))

            def z_of(xp):
                return xp[:].rearrange("p (ib c) -> p ib c", c=D1)[
                    :, :, DK:D1].rearrange("p ib one -> p (ib one)")

            # norm: xsum = lo(SBUF) + hi(PSUM); muls split DVE/ACT
            zs = npool.tile([P, NIB], f32, tag="zeps")
            nc.vector.tensor_tensor(out=zs[:, 0:4], in0=z_of(lo_sb)[:, 0:4],
                                    in1=z_of(hia), op=mybir.AluOpType.add)
            nc.vector.tensor_tensor(out=zs[:, 4:8], in0=z_of(lo_sb)[:, 4:8],
                                    in1=z_of(hib), op=mybir.AluOpType.add)
            ze = npool.tile([P, NIB], f32, tag="rz")
            nc.vector.tensor_scalar_add(ze[:], zs[:], EPS)
            rz7 = npool.tile([P, NIB], f32, tag="zeps")
            nc.vector.reciprocal(rz7[:], ze[:])
            xn27 = get_xn2(h7 // 2, su7)
            xs7 = npool.tile([P, NIB, DK], f32, tag="xs7")

            def norm_half(ib_lo):
                for ib in range(ib_lo, ib_lo + 4):
                    hi = hia if ib < 4 else hib
                    ib2 = ib % 4
                    nc.vector.tensor_add(
                        xs7[:, ib, :], lo_sb[:, ib * D1:ib * D1 + DK],
                        hi[:, ib2 * D1:ib2 * D1 + DK],
                    )
                    if ib % 2 == 1:
                        nc.scalar.mul(xn27[:, ib, h7 % 2, :], xs7[:, ib, :],
                                      rz7[:, ib:ib + 1])
                    else:
                        nc.vector.tensor_scalar_mul(
                            xn27[:, ib, h7 % 2, :], xs7[:, ib, :],
                            rz7[:, ib:ib + 1])

            norm_half(0)
            norm_half(4)
            emit_tp(h7 // 2, su7, xn27, ib_lo=0, nib=4)
            for m2 in range(F // P):
                op_group(m2, 1, 0, eng_act=(m2 % 2 == 1))
            emit_tp(h7 // 2, su7, xn27, ib_lo=4, nib=4)
            for m2 in range(F // P):
                op_group(m2, 1, 1, eng_act=(m2 % 2 == 1))

    nc.compile()
    return nc


def _prep_in_maps(query, key, value, mask, Wq, bq, Wk, bk, Wv, bv, Wo,
                  mask_ones):
    ident = np.eye(P, dtype=np.float32).astype(BF)
    B = query.shape[0]
    xTs = {}
    for b in range(B):
        m01 = (mask[b, 0, :] != 0)
        xv_full = value[b] * m01[:, None].astype(np.float32)
        xTs[b] = (
            np.ascontiguousarray(key[b].T).astype(BF),
            np.ascontiguousarray(query[b].T).astype(BF),
            np.ascontiguousarray(xv_full.T).astype(BF),
            np.ascontiguousarray(
                m01.astype(np.float32).reshape(NT, P).T) if not mask_ones
            else None,
        )
    in_maps = []
    for c in range(8):
        b = c // 2
        hh = c % 2
        ob = slice(hh * OB, (hh + 1) * OB)
        xkT, xqT, xvT, mc = xTs[b]
        bqk_h = np.concatenate(
            [bk[ob].reshape(OB // P, P).T, bq[ob].reshape(OB // P, P).T],
            axis=1,
        )
        wkqv_h = np.stack([
            np.ascontiguousarray(Wk[ob, :].T),
            np.ascontiguousarray(Wq[ob, :].T),
            np.ascontiguousarray(Wv[ob, :].T),
        ]).astype(BF)
        m = {
            "xk": xkT,
            "xq": xqT,
            "xv": xvT,
            "wkqv": wkqv_h,
            "wo": np.ascontiguousarray(Wo[:, ob].T).astype(BF),
            "bqk": np.ascontiguousarray(bqk_h),
            "bvb": np.ascontiguousarray(np.tile(bv[ob][None, :], (P, 1))),
            "ident": ident,
        }
        if not mask_ones:
            m["mcol"] = mc
        in_maps.append(m)
    return in_maps


def kernel(query, key, value, mask, Wq, bq, Wk, bk, Wv, bv, Wo, bo):
    query = np.asarray(query, dtype=np.float32)
    key = np.asarray(key, dtype=np.float32)
    value = np.asarray(value, dtype=np.float32)
    mask = np.asarray(mask)
    Wq = np.asarray(Wq, dtype=np.float32)
    bq = np.asarray(bq, dtype=np.float32)
    Wk = np.asarray(Wk, dtype=np.float32)
    bk = np.asarray(bk, dtype=np.float32)
    Wv = np.asarray(Wv, dtype=np.float32)
    bv = np.asarray(bv, dtype=np.float32)
    Wo = np.asarray(Wo, dtype=np.float32)
    bo = np.asarray(bo, dtype=np.float32)

    mask_ones = bool(np.all(mask != 0))
    ckey = ("nc", mask_ones)
    if ckey not in _CACHE:
        _CACHE[ckey] = _build(mask_ones)
        _CACHE["nc"] = _CACHE[ckey]  # test.py reads _CACHE["nc"]
    nc = _CACHE[ckey]

    B = query.shape[0]
    in_maps = _prep_in_maps(
        query, key, value, mask, Wq, bq, Wk, bk, Wv, bv, Wo, mask_ones
    )
    res = run_bass_kernel_spmd(nc, in_maps, core_ids=list(range(8)))

    out = np.empty((B, T, F), dtype=np.float32)
    for b in range(B):
        acc = res.results[2 * b]["outT"] + res.results[2 * b + 1]["outT"]
        out[b] = acc.T + bo[None, :]
    return out


# revision 13
# speedup vs baseline: 1.4855x; 1.0016x over previous
"""Multi-head attention Trainium2 kernel (8 NeuronCores, SPMD), v4.

Problem: B=4, T=2048, n_feat=512, H=8 heads, d_k=64.
Sharding: core c -> batch b = c//2, head-half hh = c%2 (4 heads = 256 attn dims).

Design (ACT-exp-bound: 128 exps of [128,1024] = ~133us on the scalar engine is
the floor; everything else hides under it):
- Host pre-transposes + bf16-casts activations (x^T [512,2048]) and weights.
- Q^T/K^T projections in [o, t] layout, V in [t, o] layout with a ones column
  (softmax denominator falls out of the PV matmul).
- Scores S^T[j, i] per (head, i-super of 1024); exp on ACT, bf16 out.
- PV out[i-block(128), dk+1] with et stationary. TRN2 psum accumulation must
  be one contiguous run per region (interleaved/reopened groups silently drop
  a visit), so PV for pair p runs ib-major (16 consecutive matmuls per
  region) interleaved into pair p+1's jt loop; all 16 et tiles of a pair stay
  live in SBUF.
- Norm = reciprocal + per-partition scalar mul (denominator is psum col 64).
- Normalized x transposed on PE in 64x64 quadrants (tile_position) so both
  heads of a pair-half pack onto 128 partitions -> output projection
  contracts 2x128 instead of 4x64 (half the matmuls).
- Tail: last pair's PV split into jt-halves (lo into xpa/xpb, hi into pp-tag
  psum), norm/transpose/outproj pipelined per i-half, ACT helps after the exp
  stream ends. Head: packed DMA prefix + PE warmup to beat the pstate ramp.
Host sums the two head-half partials per batch, transposes, adds bo.
"""
import sys

sys.path.insert(0, "/opt/trn_rl_repo")

import numpy as np
import ml_dtypes

import concourse.bass as bass
import concourse.tile as tile
from concourse import bacc, mybir
from concourse.bass_utils import run_bass_kernel_spmd

P = 128
T = 2048
F = 512            # n_feat (projection contraction dim)
OB = 256           # per-core attention dims (4 heads x 64)
NH = 4             # local heads
DK = 64
NT = T // P        # 16 row tiles
FO = F // P        # 4 feature tiles
NSUP = 2           # i-supers per head
ISUP = T // NSUP   # 1024
NIB = ISUP // P    # 8 i-blocks per super
JT = NT            # 16 j tiles
EPS = 1e-8
D1 = DK + 1

f32 = mybir.dt.float32
bf16 = mybir.dt.bfloat16
BF = ml_dtypes.bfloat16
Exp = mybir.ActivationFunctionType.Exp

_CACHE = {}


def _build(mask_ones: bool):
    nc = bacc.Bacc("TRN2", target_bir_lowering=False, debug=False, num_devices=8)

    xk = nc.dram_tensor("xk", (F, T), bf16, kind="ExternalInput").ap()
    xq = nc.dram_tensor("xq", (F, T), bf16, kind="ExternalInput").ap()
    xv = nc.dram_tensor("xv", (F, T), bf16, kind="ExternalInput").ap()
    wkq = nc.dram_tensor("wkq", (2, F, OB), bf16, kind="ExternalInput").ap()
    wvd = nc.dram_tensor("wvd", (F, OB), bf16, kind="ExternalInput").ap()
    wo = nc.dram_tensor("wo", (OB, F), bf16, kind="ExternalInput").ap()
    bqk = nc.dram_tensor("bqk", (P, 4), f32, kind="ExternalInput").ap()
    bvb = nc.dram_tensor("bvb", (P, OB), f32, kind="ExternalInput").ap()
    ident = nc.dram_tensor("ident", (P, P), bf16, kind="ExternalInput").ap()
    if not mask_ones:
        mcol = nc.dram_tensor("mcol", (P, NT), f32, kind="ExternalInput").ap()
    outT = nc.dram_tensor("outT", (F, T), f32, kind="ExternalOutput").ap()

    xk_r = xk.rearrange("(fo p) t -> p fo t", p=P)
    xq_r = xq.rearrange("(fo p) t -> p fo t", p=P)
    xv_r = xv.rearrange("(fo p) t -> p fo t", p=P)

    with tile.TileContext(nc) as tc:
        with tc.tile_pool(name="const", bufs=1) as cpool, \
             tc.tile_pool(name="act", bufs=1) as apool, \
             tc.tile_pool(name="persist", bufs=1) as ppool, \
             tc.tile_pool(name="et", bufs=26) as epool, \
             tc.tile_pool(name="norm", bufs=2) as npool, \
             tc.tile_pool(name="out", bufs=4) as opool, \
             tc.tile_pool(name="ps_st", bufs=2, space="PSUM") as ps_st, \
             tc.tile_pool(name="ps_pp", bufs=2, space="PSUM") as ps_pp, \
             tc.tile_pool(name="ps_xp", bufs=1, space="PSUM") as ps_xp:

            # ---- SBUF tiles ----
            wkqv_sb = cpool.tile([P, 3, FO, OB], bf16, tag="wkqv")
            wo_sb = cpool.tile([P, 2, F], bf16, tag="wo")
            bqk_sb = cpool.tile([P, 4], f32, tag="bqk")
            bv_sb = cpool.tile([P, OB], f32, tag="bv")
            id_sb = cpool.tile([P, P], bf16, tag="ident")
            ws = cpool.tile([P, OB], bf16, tag="warm")
            if not mask_ones:
                mc_sb = cpool.tile([P, NT], f32, tag="mcol")
                z3_sb = cpool.tile([P, NH, 1], f32, tag="z3")

            xk_sb = apool.tile([P, FO, T], bf16, tag="xk")
            xq_sb = apool.tile([P, FO, T], bf16, tag="xq")
            xv_sb = apool.tile([P, FO, T], bf16, tag="xv")

            KT = ppool.tile([P, OB // P, T], bf16, tag="KT")
            QT = ppool.tile([P, OB // P, T], bf16, tag="QT")
            V2 = ppool.tile([P, NT, NH, D1], bf16, tag="V2")
            xT2 = ppool.tile([P, 2, T], bf16, tag="xT2")

            wk_sb = wkqv_sb[:, 0, :, :]
            wq_sb = wkqv_sb[:, 1, :, :]
            wv_sb = wkqv_sb[:, 2, :, :]
            bk_col = bqk_sb[:, 0:2]
            bq_col = bqk_sb[:, 2:4]

            # ---- PE warmup: keep PE busy through the pstate ramp ----
            nc.vector.memset(ws[:], 0.01)
            for wi in range(26):
                wp = ps_st.tile([P, OB], f32, tag="st", name=f"warm_{wi}")
                nc.tensor.matmul(
                    wp[:, :OB], ws[:, 0:P], ws[:, 0:OB], start=True, stop=True
                )

            # ---- DMA plan (single SP queue, prioritized order) ----
            def dma_x(xr, dst, c):
                nc.sync.dma_start(
                    out=dst[:, :, c * F:(c + 1) * F], in_=xr[:, :, c * F:(c + 1) * F]
                )

            nc.sync.dma_start(out=bqk_sb[:], in_=bqk[:])
            nc.sync.dma_start(
                out=wkqv_sb[:, 0:2, :, :],
                in_=wkq.rearrange("w (fo p) o -> p w fo o", p=P),
            )
            dma_x(xk_r, xk_sb, 0)
            dma_x(xq_r, xq_sb, 0)
            dma_x(xq_r, xq_sb, 1)
            nc.sync.dma_start(
                out=wkqv_sb[:, 2, :, :],
                in_=wvd.rearrange("(fo p) o -> p fo o", p=P),
            )
            nc.sync.dma_start(out=bv_sb[:], in_=bvb[:])
            dma_x(xv_r, xv_sb, 0)
            nc.sync.dma_start(out=id_sb[:], in_=ident[:])
            if not mask_ones:
                nc.sync.dma_start(out=mc_sb[:], in_=mcol[:])
            dma_x(xk_r, xk_sb, 1)
            dma_x(xv_r, xv_sb, 1)
            dma_x(xk_r, xk_sb, 2)
            dma_x(xv_r, xv_sb, 2)
            dma_x(xk_r, xk_sb, 3)
            dma_x(xv_r, xv_sb, 3)
            dma_x(xq_r, xq_sb, 2)
            dma_x(xq_r, xq_sb, 3)
            nc.sync.dma_start(out=wo_sb[:], in_=wo.rearrange("(oh p) f -> p oh f", p=P))

            # V2 ones column (or mask column)
            nc.vector.memset(V2[:, :, :, DK:D1], 1.0)
            if not mask_ones:
                nc.vector.memset(z3_sb[:], 0.0)

            # ---- projection emitters ----
            def qk_chunk(w_sb, b_col, dst, x_sb, po, c):
                pp = ps_pp.tile([P, F], f32, tag="pp", name=f"qk_{po}_{c}")
                for fo in range(FO):
                    nc.tensor.matmul(
                        pp[:, :F],
                        w_sb[:, fo, po * P:(po + 1) * P],
                        x_sb[:, fo, c * F:(c + 1) * F],
                        start=(fo == 0),
                        stop=(fo == FO - 1),
                    )
                nc.vector.tensor_scalar_add(
                    dst[:, po, c * F:(c + 1) * F], pp[:, :F], b_col[:, po:po + 1]
                )

            def v_chunk(tb):
                pp = ps_pp.tile([P, F], f32, tag="pp", name=f"v_{tb}")
                for fo in range(FO):
                    nc.tensor.matmul(
                        pp[:, :OB],
                        xv_sb[:, fo, tb * P:(tb + 1) * P],
                        wv_sb[:, fo, :],
                        start=(fo == 0),
                        stop=(fo == FO - 1),
                    )
                if mask_ones:
                    nc.vector.tensor_add(
                        V2[:, tb, :, 0:DK],
                        pp[:, :OB].rearrange("p (h d) -> p h d", h=NH),
                        bv_sb[:].rearrange("p (h d) -> p h d", h=NH),
                    )
                else:
                    vt = npool.tile([P, NH, DK], f32, tag="vt")
                    nc.vector.tensor_add(
                        vt[:],
                        pp[:, :OB].rearrange("p (h d) -> p h d", h=NH),
                        bv_sb[:].rearrange("p (h d) -> p h d", h=NH),
                    )
                    nc.vector.tensor_scalar_mul(
                        V2[:, tb, :, 0:DK], vt[:], mc_sb[:, tb:tb + 1]
                    )
                    nc.vector.tensor_scalar_add(
                        V2[:, tb, :, DK:D1], z3_sb[:], mc_sb[:, tb:tb + 1]
                    )

            # ---- head start: K po0 c0, Q po0 c0-c1 ----
            qk_chunk(wk_sb, bk_col, KT, xk_sb, 0, 0)
            qk_chunk(wq_sb, bq_col, QT, xq_sb, 0, 0)
            qk_chunk(wq_sb, bq_col, QT, xq_sb, 0, 1)
            v_chunk(0)
            v_chunk(1)

            # ---- norm / transpose / outproj emitters ----
            def emit_norm_mul(xn, ib, src_ap, rz, eng_act=False):
                if eng_act:
                    nc.scalar.mul(xn[:, ib, :], src_ap, rz[:, ib:ib + 1])
                else:
                    nc.vector.tensor_scalar_mul(xn[:, ib, :], src_ap,
                                                rz[:, ib:ib + 1])

            xn2_hold = {}

            def get_xn2(oh, su):
                if (oh, su) not in xn2_hold:
                    xn2_hold[(oh, su)] = npool.tile(
                        [P, NIB, 2, DK], bf16, tag="xn",
                        name=f"xn2_{oh}_{su}")
                return xn2_hold[(oh, su)]

            def emit_norm(pi, h, su, xpa, xpb):
                zeps = npool.tile([P, NIB], f32, tag="zeps")
                za = xpa[:].rearrange("p (ib c) -> p ib c", c=D1)[
                    :, :, DK:D1].rearrange("p ib one -> p (ib one)")
                zb = xpb[:].rearrange("p (ib c) -> p ib c", c=D1)[
                    :, :, DK:D1].rearrange("p ib one -> p (ib one)")
                nc.vector.tensor_scalar_add(zeps[:, 0:4], za, EPS)
                nc.vector.tensor_scalar_add(zeps[:, 4:8], zb, EPS)
                rz = npool.tile([P, NIB], f32, tag="rz")
                nc.vector.reciprocal(rz[:], zeps[:])
                xn2 = get_xn2(h // 2, su)
                for ib in range(NIB):
                    src = xpa if ib < 4 else xpb
                    ib2 = ib % 4
                    nc.vector.tensor_scalar_mul(
                        xn2[:, ib, h % 2, :],
                        src[:, ib2 * D1:ib2 * D1 + DK],
                        rz[:, ib:ib + 1],
                    )
                return xn2

            def emit_tp_ib(tp, xn2, ib):
                # full 128x128 transpose: [i, (h d)] -> [(h d), i]
                nc.tensor.transpose(
                    tp[:, ib * P:(ib + 1) * P],
                    xn2[:, ib, :, :].rearrange("p h d -> p (h d)"),
                    id_sb[:],
                )

            def emit_tp(oh, su, xn2, ib_lo=0, nib=NIB):
                tp = ps_pp.tile([P, ISUP], bf16, tag="pp",
                                name=f"tp_{oh}_{su}_{ib_lo}")
                for ib in range(ib_lo, ib_lo + nib):
                    emit_tp_ib(tp, xn2, ib)
                nc.vector.tensor_copy(
                    xT2[:, oh, su * ISUP + ib_lo * P:
                        su * ISUP + (ib_lo + nib) * P],
                    tp[:, ib_lo * P:(ib_lo + nib) * P],
                )

            os2_hold = {}

            def op_group(m2, su, cc, eng_act=False):
                if cc == 0:
                    os2_hold[m2] = opool.tile([P, 2, F], f32, tag="os2",
                                              name=f"os2_{su}_{m2}")
                os2 = os2_hold[m2]
                c = su * 2 + cc
                pp = ps_pp.tile([P, F], f32, tag="pp", name=f"op_{m2}_{c}")
                for oh in range(2):
                    nc.tensor.matmul(
                        pp[:, :F],
                        wo_sb[:, oh, m2 * P:(m2 + 1) * P],
                        xT2[:, oh, c * F:(c + 1) * F],
                        start=(oh == 0),
                        stop=(oh == 1),
                    )
                if eng_act:
                    nc.scalar.copy(os2[:, cc, :], pp[:, :F])
                else:
                    nc.vector.tensor_copy(os2[:, cc, :], pp[:, :F])
                if cc == 1:
                    nc.sync.dma_start(
                        out=outT[m2 * P:(m2 + 1) * P, su * ISUP:(su + 1) * ISUP],
                        in_=os2[:].rearrange("p c f -> p (c f)"),
                    )

            # ---- per-pair interleaved extras ----
            def K_(po, c):
                return lambda: qk_chunk(wk_sb, bk_col, KT, xk_sb, po, c)

            def Q_(po, c):
                return lambda: qk_chunk(wq_sb, bq_col, QT, xq_sb, po, c)

            def OP_(m2, su, cc):
                return lambda: op_group(m2, su, cc)

            extras_by_pair = {
                0: {1: [K_(0, 1)], 2: [K_(0, 2)], 3: [K_(0, 3)]},
                1: {9: [K_(1, 0)], 10: [K_(1, 1)], 11: [K_(1, 2)],
                    12: [K_(1, 3)], 13: [Q_(1, 0)], 14: [Q_(1, 1)]},
                2: {8: [Q_(1, 2)], 10: [Q_(1, 3)], 12: [Q_(0, 2)],
                    14: [Q_(0, 3)]},
                5: {3: [OP_(0, 0, 0)], 5: [OP_(0, 0, 1)], 7: [OP_(1, 0, 0)],
                    9: [OP_(1, 0, 1)]},
                6: {1: [OP_(2, 0, 0)], 3: [OP_(2, 0, 1)], 5: [OP_(3, 0, 0)],
                    7: [OP_(3, 0, 1)]},
            }

            # ---- attention pairs ----
            # PV for pair p: 8 ib-major runs (16 consecutive matmuls each)
            # interleaved into pair p+1's jt loop. Last pair: jt-halves.
            pairs = [(su, h) for su in range(NSUP) for h in range(NH)]
            LAST = len(pairs) - 1
            state = {}

            def pv_run(p, ib, jt0=0, njt=JT, tgt_pair=None, fresh=True):
                ps = state[p]
                if tgt_pair is None:
                    if ib == 0 and fresh:
                        ps["xpa"] = ps_xp.tile([P, 4 * D1], f32, tag="xpa",
                                               name=f"xpa_{p}")
                        ps["xpb"] = ps_xp.tile([P, 4 * D1], f32, tag="xpb",
                                               name=f"xpb_{p}")
                    tgt = ps["xpa"] if ib < 4 else ps["xpb"]
                else:
                    tgt = tgt_pair[0] if ib < 4 else tgt_pair[1]
                ib2 = ib % 4
                hp = ps["h"]
                for jj in range(njt):
                    jt = jt0 + jj
                    nc.tensor.matmul(
                        tgt[:, ib2 * D1:(ib2 + 1) * D1],
                        ps["et"][jt][:, ib * P:(ib + 1) * P],
                        V2[:, jt, hp, :],
                        start=(jj == 0),
                        stop=(jj == njt - 1),
                    )

            def drain_pair(p):
                ps = state[p]
                h, su = ps["h"], ps["su"]
                xn2 = emit_norm(p, h, su, ps["xpa"], ps["xpb"])
                if h % 2 == 1:
                    emit_tp(h // 2, su, xn2)
                    del xn2_hold[(h // 2, su)]
                del state[p]

            for pi, (su, h) in enumerate(pairs):
                qoff = (h % 2) * DK
                qpo = h // 2
                isl = su * ISUP
                extras = extras_by_pair.get(pi, {})
                state[pi] = {"h": h, "su": su, "et": []}

                def scores(jt):
                    st = ps_st.tile([P, ISUP], f32, tag="st",
                                    name=f"st_{pi}_{jt}")
                    for c2 in range(ISUP // F):
                        nc.tensor.matmul(
                            st[:, c2 * F:(c2 + 1) * F],
                            KT[qoff:qoff + DK, qpo, jt * P:(jt + 1) * P],
                            QT[qoff:qoff + DK, qpo,
                               isl + c2 * F:isl + (c2 + 1) * F],
                            start=True,
                            stop=True,
                        )
                    return st

                if pi == 0:
                    # first scores tile: c0 half only, so the exp stream can
                    # start as soon as QT c0 lands; c1 finishes inside jt0
                    st_prev = ps_st.tile([P, ISUP], f32, tag="st",
                                         name="st_0_0")
                    nc.tensor.matmul(
                        st_prev[:, 0:F],
                        KT[qoff:qoff + DK, qpo, 0:P],
                        QT[qoff:qoff + DK, qpo, isl:isl + F],
                        start=True, stop=True,
                    )
                else:
                    st_prev = scores(0)
                for jt in range(JT):
                    et = epool.tile([P, ISUP], bf16, tag="et",
                                    name=f"et_{pi}_{jt}")
                    if pi == 0 and jt == 0:
                        nc.scalar.activation(et[:, 0:F], st_prev[:, 0:F],
                                             Exp, scale=0.125)
                        nc.tensor.matmul(
                            st_prev[:, F:ISUP],
                            KT[qoff:qoff + DK, qpo, 0:P],
                            QT[qoff:qoff + DK, qpo, isl + F:isl + ISUP],
                            start=True, stop=True,
                        )
                        nc.scalar.activation(et[:, F:ISUP], st_prev[:, F:ISUP],
                                             Exp, scale=0.125)
                    else:
                        nc.scalar.activation(et[:], st_prev[:], Exp,
                                             scale=0.125)
                    state[pi]["et"].append(et)
                    if jt + 1 < JT:
                        st_prev = scores(jt + 1)
                    if pi == 0 and jt + 2 < NT:
                        v_chunk(jt + 2)
                    if pi > 0 and jt < NIB:
                        pv_run(pi - 1, jt)
                    if pi > 0 and jt == NIB:
                        drain_pair(pi - 1)
                    if pi == LAST and jt >= NIB:
                        # lo-half runs of the last pair (jts 0-7)
                        pv_run(LAST, jt - NIB, jt0=0, njt=NIB)
                    for fn in extras.get(jt, []):
                        fn()

            # ---- tail: last pair hi-half + norm/tp/outproj per i-half ----
            ps7 = state[LAST]
            h7, su7 = ps7["h"], ps7["su"]
            # lo halves -> SBUF (overlaps the hi runs; DVE can read only one
            # PSUM operand per tensor_tensor)
            lo_sb = npool.tile([P, NIB * D1], f32, tag="losb")
            nc.vector.tensor_copy(lo_sb[:, 0:4 * D1], ps7["xpa"][:])
            nc.vector.tensor_copy(lo_sb[:, 4 * D1:NIB * D1], ps7["xpb"][:])
            hia = ps_pp.tile([P, 4 * D1], f32, tag="pp", name="hia")
            hib = ps_pp.tile([P, 4 * D1], f32, tag="pp", name="hib")
            for ib in range(NIB):
                pv_run(LAST, ib, jt0=NIB, njt=NIB, tgt_pair=(hia, hib))

            def z_of(xp):
                return xp[:].rearrange("p (ib c) -> p ib c", c=D1)[
                    :, :, DK:D1].rearrange("p ib one -> p (ib one)")

            # norm: xsum = lo(SBUF) + hi(PSUM); muls split DVE/ACT
            zs = npool.tile([P, NIB], f32, tag="zeps")
            nc.vector.tensor_tensor(out=zs[:, 0:4], in0=z_of(lo_sb)[:, 0:4],
                                    in1=z_of(hia), op=mybir.AluOpType.add)
            nc.vector.tensor_tensor(out=zs[:, 4:8], in0=z_of(lo_sb)[:, 4:8],
                                    in1=z_of(hib), op=mybir.AluOpType.add)
            ze = npool.tile([P, NIB], f32, tag="rz")
            nc.vector.tensor_scalar_add(ze[:], zs[:], EPS)
            rz7 = npool.tile([P, NIB], f32, tag="zeps")
            nc.vector.reciprocal(rz7[:], ze[:])
            xn27 = get_xn2(h7 // 2, su7)
            xs7 = npool.tile([P, NIB, DK], f32, tag="xs7")

            def norm_half(ib_lo):
                for ib in range(ib_lo, ib_lo + 4):
                    hi = hia if ib < 4 else hib
                    ib2 = ib % 4
                    nc.vector.tensor_add(
                        xs7[:, ib, :], lo_sb[:, ib * D1:ib * D1 + DK],
                        hi[:, ib2 * D1:ib2 * D1 + DK],
                    )
                    if ib % 2 == 1:
                        nc.scalar.mul(xn27[:, ib, h7 % 2, :], xs7[:, ib, :],
                                      rz7[:, ib:ib + 1])
                    else:
                        nc.vector.tensor_scalar_mul(
                            xn27[:, ib, h7 % 2, :], xs7[:, ib, :],
                            rz7[:, ib:ib + 1])

            norm_half(0)
            norm_half(4)
            emit_tp(h7 // 2, su7, xn27, ib_lo=0, nib=4)
            for m2 in range(F // P):
                op_group(m2, 1, 0, eng_act=(m2 % 2 == 1))
            emit_tp(h7 // 2, su7, xn27, ib_lo=4, nib=4)
            for m2 in range(F // P):
                op_group(m2, 1, 1, eng_act=(m2 % 2 == 1))

    nc.compile()
    return nc


def _prep_in_maps(query, key, value, mask, Wq, bq, Wk, bk, Wv, bv, Wo,
                  mask_ones):
    ident = np.eye(P, dtype=np.float32).astype(BF)
    B = query.shape[0]
    xTs = {}
    for b in range(B):
        m01 = (mask[b, 0, :] != 0)
        xv_full = value[b] * m01[:, None].astype(np.float32)
        xTs[b] = (
            np.ascontiguousarray(key[b].T).astype(BF),
            np.ascontiguousarray(query[b].T).astype(BF),
            np.ascontiguousarray(xv_full.T).astype(BF),
            np.ascontiguousarray(
                m01.astype(np.float32).reshape(NT, P).T) if not mask_ones
            else None,
        )
    in_maps = []
    for c in range(8):
        b = c // 2
        hh = c % 2
        ob = slice(hh * OB, (hh + 1) * OB)
        xkT, xqT, xvT, mc = xTs[b]
        bqk_h = np.concatenate(
            [bk[ob].reshape(OB // P, P).T, bq[ob].reshape(OB // P, P).T],
            axis=1,
        )
        wkq_h = np.stack([
            np.ascontiguousarray(Wk[ob, :].T),
            np.ascontiguousarray(Wq[ob, :].T),
        ]).astype(BF)
        wv_h = np.ascontiguousarray(Wv[ob, :].T).astype(BF)
        m = {
            "xk": xkT,
            "xq": xqT,
            "xv": xvT,
            "wkq": wkq_h,
            "wvd": wv_h,
            "wo": np.ascontiguousarray(Wo[:, ob].T).astype(BF),
            "bqk": np.ascontiguousarray(bqk_h),
            "bvb": np.ascontiguousarray(np.tile(bv[ob][None, :], (P, 1))),
            "ident": ident,
        }
        if not mask_ones:
            m["mcol"] = mc
        in_maps.append(m)
    return in_maps


def kernel(query, key, value, mask, Wq, bq, Wk, bk, Wv, bv, Wo, bo):
    query = np.asarray(query, dtype=np.float32)
    key = np.asarray(key, dtype=np.float32)
    value = np.asarray(value, dtype=np.float32)
    mask = np.asarray(mask)
    Wq = np.asarray(Wq, dtype=np.float32)
    bq = np.asarray(bq, dtype=np.float32)
    Wk = np.asarray(Wk, dtype=np.float32)
    bk = np.asarray(bk, dtype=np.float32)
    Wv = np.asarray(Wv, dtype=np.float32)
    bv = np.asarray(bv, dtype=np.float32)
    Wo = np.asarray(Wo, dtype=np.float32)
    bo = np.asarray(bo, dtype=np.float32)

    mask_ones = bool(np.all(mask != 0))
    ckey = ("nc", mask_ones)
    if ckey not in _CACHE:
        _CACHE[ckey] = _build(mask_ones)
        _CACHE["nc"] = _CACHE[ckey]  # test.py reads _CACHE["nc"]
    nc = _CACHE[ckey]

    B = query.shape[0]
    in_maps = _prep_in_maps(
        query, key, value, mask, Wq, bq, Wk, bk, Wv, bv, Wo, mask_ones
    )
    res = run_bass_kernel_spmd(nc, in_maps, core_ids=list(range(8)))

    out = np.empty((B, T, F), dtype=np.float32)
    for b in range(B):
        acc = res.results[2 * b]["outT"] + res.results[2 * b + 1]["outT"]
        out[b] = acc.T + bo[None, :]
    return out


# revision 14
# speedup vs baseline: 1.5239x; 1.0259x over previous
"""Multi-head attention Trainium2 kernel (8 NeuronCores, SPMD), v4.

Problem: B=4, T=2048, n_feat=512, H=8 heads, d_k=64.
Sharding: core c -> batch b = c//2, head-half hh = c%2 (4 heads = 256 attn dims).

Design (ACT-exp-bound: 128 exps of [128,1024] = ~133us on the scalar engine is
the floor; everything else hides under it):
- Host pre-transposes + bf16-casts activations (x^T [512,2048]) and weights.
- Q^T/K^T projections in [o, t] layout, V in [t, o] layout with a ones column
  (softmax denominator falls out of the PV matmul).
- Scores S^T[j, i] per (head, i-super of 1024); exp on ACT, bf16 out.
- PV out[i-block(128), dk+1] with et stationary. TRN2 psum accumulation must
  be one contiguous run per region (interleaved/reopened groups silently drop
  a visit), so PV for pair p runs ib-major (16 consecutive matmuls per
  region) interleaved into pair p+1's jt loop; all 16 et tiles of a pair stay
  live in SBUF.
- Norm = reciprocal + per-partition scalar mul (denominator is psum col 64).
- Normalized x transposed on PE in 64x64 quadrants (tile_position) so both
  heads of a pair-half pack onto 128 partitions -> output projection
  contracts 2x128 instead of 4x64 (half the matmuls).
- Tail: last pair's PV split into jt-halves (lo into xpa/xpb, hi into pp-tag
  psum), norm/transpose/outproj pipelined per i-half, ACT helps after the exp
  stream ends. Head: packed DMA prefix + PE warmup to beat the pstate ramp.
Host sums the two head-half partials per batch, transposes, adds bo.
"""
import sys

sys.path.insert(0, "/opt/trn_rl_repo")

import numpy as np
import ml_dtypes

import concourse.bass as bass
import concourse.tile as tile
from concourse import bacc, mybir
from concourse.bass_utils import run_bass_kernel_spmd

P = 128
T = 2048
F = 512            # n_feat (projection contraction dim)
OB = 256           # per-core attention dims (4 heads x 64)
NH = 4             # local heads
DK = 64
NT = T // P        # 16 row tiles
FO = F // P        # 4 feature tiles
NSUP = 2           # i-supers per head
ISUP = T // NSUP   # 1024
NIB = ISUP // P    # 8 i-blocks per super
JT = NT            # 16 j tiles
EPS = 1e-8
D1 = DK + 1

f32 = mybir.dt.float32
bf16 = mybir.dt.bfloat16
BF = ml_dtypes.bfloat16
Exp = mybir.ActivationFunctionType.Exp

_CACHE = {}


def _build(mask_ones: bool):
    nc = bacc.Bacc("TRN2", target_bir_lowering=False, debug=False, num_devices=8)

    xk = nc.dram_tensor("xk", (F, T), bf16, kind="ExternalInput").ap()
    xq = nc.dram_tensor("xq", (F, T), bf16, kind="ExternalInput").ap()
    xv = nc.dram_tensor("xv", (F, T), bf16, kind="ExternalInput").ap()
    wkq = nc.dram_tensor("wkq", (2, F, OB), bf16, kind="ExternalInput").ap()
    wvd = nc.dram_tensor("wvd", (F, OB), bf16, kind="ExternalInput").ap()
    wo = nc.dram_tensor("wo", (OB, F), bf16, kind="ExternalInput").ap()
    bqk = nc.dram_tensor("bqk", (P, 4), f32, kind="ExternalInput").ap()
    bvb = nc.dram_tensor("bvb", (P, OB), f32, kind="ExternalInput").ap()
    ident = nc.dram_tensor("ident", (P, P), bf16, kind="ExternalInput").ap()
    if not mask_ones:
        mcol = nc.dram_tensor("mcol", (P, NT), f32, kind="ExternalInput").ap()
    outT = nc.dram_tensor("outT", (F, T), f32, kind="ExternalOutput").ap()

    xk_r = xk.rearrange("(fo p) t -> p fo t", p=P)
    xq_r = xq.rearrange("(fo p) t -> p fo t", p=P)
    xv_r = xv.rearrange("(fo p) t -> p fo t", p=P)

    with tile.TileContext(nc) as tc:
        with tc.tile_pool(name="const", bufs=1) as cpool, \
             tc.tile_pool(name="act", bufs=1) as apool, \
             tc.tile_pool(name="persist", bufs=1) as ppool, \
             tc.tile_pool(name="et", bufs=26) as epool, \
             tc.tile_pool(name="norm", bufs=2) as npool, \
             tc.tile_pool(name="out", bufs=4) as opool, \
             tc.tile_pool(name="ps_st", bufs=2, space="PSUM") as ps_st, \
             tc.tile_pool(name="ps_pp", bufs=2, space="PSUM") as ps_pp, \
             tc.tile_pool(name="ps_xp", bufs=1, space="PSUM") as ps_xp:

            # ---- SBUF tiles ----
            wkqv_sb = cpool.tile([P, 3, FO, OB], bf16, tag="wkqv")
            wo_sb = cpool.tile([P, 2, F], bf16, tag="wo")
            bqk_sb = cpool.tile([P, 4], f32, tag="bqk")
            bv_sb = cpool.tile([P, OB], f32, tag="bv")
            id_sb = cpool.tile([P, P], bf16, tag="ident")
            ws = cpool.tile([P, OB], bf16, tag="warm")
            if not mask_ones:
                mc_sb = cpool.tile([P, NT], f32, tag="mcol")
                z3_sb = cpool.tile([P, NH, 1], f32, tag="z3")

            xk_sb = apool.tile([P, FO, T], bf16, tag="xk")
            xq_sb = apool.tile([P, FO, T], bf16, tag="xq")
            xv_sb = apool.tile([P, FO, T], bf16, tag="xv")

            KT = ppool.tile([P, OB // P, T], bf16, tag="KT")
            QT = ppool.tile([P, OB // P, T], bf16, tag="QT")
            V2 = ppool.tile([P, NT, NH, D1], bf16, tag="V2")
            xT2 = ppool.tile([P, 2, T], bf16, tag="xT2")

            wk_sb = wkqv_sb[:, 0, :, :]
            wq_sb = wkqv_sb[:, 1, :, :]
            wv_sb = wkqv_sb[:, 2, :, :]
            bk_col = bqk_sb[:, 0:2]
            bq_col = bqk_sb[:, 2:4]

            # ---- PE warmup: keep PE busy through the pstate ramp ----
            nc.vector.memset(ws[:], 0.01)
            for wi in range(26):
                wp = ps_st.tile([P, OB], f32, tag="st", name=f"warm_{wi}")
                nc.tensor.matmul(
                    wp[:, :OB], ws[:, 0:P], ws[:, 0:OB], start=True, stop=True
                )

            # ---- DMA plan (single SP queue, prioritized order) ----
            def dma_x(xr, dst, c):
                nc.sync.dma_start(
                    out=dst[:, :, c * F:(c + 1) * F], in_=xr[:, :, c * F:(c + 1) * F]
                )

            nc.scalar.dma_start(out=bqk_sb[:], in_=bqk[:])
            nc.scalar.dma_start(out=bv_sb[:], in_=bvb[:])
            nc.scalar.dma_start(out=id_sb[:], in_=ident[:])
            if not mask_ones:
                nc.scalar.dma_start(out=mc_sb[:], in_=mcol[:])
            nc.sync.dma_start(
                out=wkqv_sb[:, 0:2, :, :],
                in_=wkq.rearrange("w (fo p) o -> p w fo o", p=P),
            )
            dma_x(xk_r, xk_sb, 0)
            dma_x(xq_r, xq_sb, 0)
            dma_x(xq_r, xq_sb, 1)
            nc.sync.dma_start(
                out=wkqv_sb[:, 2, :, :],
                in_=wvd.rearrange("(fo p) o -> p fo o", p=P),
            )
            dma_x(xv_r, xv_sb, 0)
            dma_x(xk_r, xk_sb, 1)
            dma_x(xv_r, xv_sb, 1)
            dma_x(xk_r, xk_sb, 2)
            dma_x(xv_r, xv_sb, 2)
            dma_x(xk_r, xk_sb, 3)
            dma_x(xv_r, xv_sb, 3)
            dma_x(xq_r, xq_sb, 2)
            dma_x(xq_r, xq_sb, 3)
            nc.sync.dma_start(out=wo_sb[:], in_=wo.rearrange("(oh p) f -> p oh f", p=P))

            # V2 ones column (or mask column)
            nc.vector.memset(V2[:, :, :, DK:D1], 1.0)
            if not mask_ones:
                nc.vector.memset(z3_sb[:], 0.0)

            # ---- projection emitters ----
            def qk_chunk(w_sb, b_col, dst, x_sb, po, c):
                pp = ps_pp.tile([P, F], f32, tag="pp", name=f"qk_{po}_{c}")
                for fo in range(FO):
                    nc.tensor.matmul(
                        pp[:, :F],
                        w_sb[:, fo, po * P:(po + 1) * P],
                        x_sb[:, fo, c * F:(c + 1) * F],
                        start=(fo == 0),
                        stop=(fo == FO - 1),
                    )
                nc.vector.tensor_scalar_add(
                    dst[:, po, c * F:(c + 1) * F], pp[:, :F], b_col[:, po:po + 1]
                )

            def v_chunk(tb):
                pp = ps_pp.tile([P, F], f32, tag="pp", name=f"v_{tb}")
                for fo in range(FO):
                    nc.tensor.matmul(
                        pp[:, :OB],
                        xv_sb[:, fo, tb * P:(tb + 1) * P],
                        wv_sb[:, fo, :],
                        start=(fo == 0),
                        stop=(fo == FO - 1),
                    )
                if mask_ones:
                    nc.vector.tensor_add(
                        V2[:, tb, :, 0:DK],
                        pp[:, :OB].rearrange("p (h d) -> p h d", h=NH),
                        bv_sb[:].rearrange("p (h d) -> p h d", h=NH),
                    )
                else:
                    vt = npool.tile([P, NH, DK], f32, tag="vt")
                    nc.vector.tensor_add(
                        vt[:],
                        pp[:, :OB].rearrange("p (h d) -> p h d", h=NH),
                        bv_sb[:].rearrange("p (h d) -> p h d", h=NH),
                    )
                    nc.vector.tensor_scalar_mul(
                        V2[:, tb, :, 0:DK], vt[:], mc_sb[:, tb:tb + 1]
                    )
                    nc.vector.tensor_scalar_add(
                        V2[:, tb, :, DK:D1], z3_sb[:], mc_sb[:, tb:tb + 1]
                    )

            # ---- head start: K po0 c0, Q po0 c0-c1 ----
            qk_chunk(wk_sb, bk_col, KT, xk_sb, 0, 0)
            qk_chunk(wq_sb, bq_col, QT, xq_sb, 0, 0)

            # ---- norm / transpose / outproj emitters ----
            def emit_norm_mul(xn, ib, src_ap, rz, eng_act=False):
                if eng_act:
                    nc.scalar.mul(xn[:, ib, :], src_ap, rz[:, ib:ib + 1])
                else:
                    nc.vector.tensor_scalar_mul(xn[:, ib, :], src_ap,
                                                rz[:, ib:ib + 1])

            xn2_hold = {}

            def get_xn2(oh, su):
                if (oh, su) not in xn2_hold:
                    xn2_hold[(oh, su)] = npool.tile(
                        [P, NIB, 2, DK], bf16, tag="xn",
                        name=f"xn2_{oh}_{su}")
                return xn2_hold[(oh, su)]

            def emit_norm(pi, h, su, xpa, xpb):
                zeps = npool.tile([P, NIB], f32, tag="zeps")
                za = xpa[:].rearrange("p (ib c) -> p ib c", c=D1)[
                    :, :, DK:D1].rearrange("p ib one -> p (ib one)")
                zb = xpb[:].rearrange("p (ib c) -> p ib c", c=D1)[
                    :, :, DK:D1].rearrange("p ib one -> p (ib one)")
                nc.vector.tensor_scalar_add(zeps[:, 0:4], za, EPS)
                nc.vector.tensor_scalar_add(zeps[:, 4:8], zb, EPS)
                rz = npool.tile([P, NIB], f32, tag="rz")
                nc.vector.reciprocal(rz[:], zeps[:])
                xn2 = get_xn2(h // 2, su)
                for ib in range(NIB):
                    src = xpa if ib < 4 else xpb
                    ib2 = ib % 4
                    nc.vector.tensor_scalar_mul(
                        xn2[:, ib, h % 2, :],
                        src[:, ib2 * D1:ib2 * D1 + DK],
                        rz[:, ib:ib + 1],
                    )
                return xn2

            def emit_tp_ib(tp, xn2, ib):
                # full 128x128 transpose: [i, (h d)] -> [(h d), i]
                nc.tensor.transpose(
                    tp[:, ib * P:(ib + 1) * P],
                    xn2[:, ib, :, :].rearrange("p h d -> p (h d)"),
                    id_sb[:],
                )

            def emit_tp(oh, su, xn2, ib_lo=0, nib=NIB):
                tp = ps_pp.tile([P, ISUP], bf16, tag="pp",
                                name=f"tp_{oh}_{su}_{ib_lo}")
                for ib in range(ib_lo, ib_lo + nib):
                    emit_tp_ib(tp, xn2, ib)
                nc.vector.tensor_copy(
                    xT2[:, oh, su * ISUP + ib_lo * P:
                        su * ISUP + (ib_lo + nib) * P],
                    tp[:, ib_lo * P:(ib_lo + nib) * P],
                )

            os2_hold = {}

            def op_group(m2, su, cc, eng_act=False, dma_each=False):
                if cc == 0:
                    os2_hold[m2] = opool.tile([P, 2, F], f32, tag="os2",
                                              name=f"os2_{su}_{m2}")
                os2 = os2_hold[m2]
                c = su * 2 + cc
                pp = ps_pp.tile([P, F], f32, tag="pp", name=f"op_{m2}_{c}")
                for oh in range(2):
                    nc.tensor.matmul(
                        pp[:, :F],
                        wo_sb[:, oh, m2 * P:(m2 + 1) * P],
                        xT2[:, oh, c * F:(c + 1) * F],
                        start=(oh == 0),
                        stop=(oh == 1),
                    )
                if eng_act:
                    nc.scalar.copy(os2[:, cc, :], pp[:, :F])
                else:
                    nc.vector.tensor_copy(os2[:, cc, :], pp[:, :F])
                if dma_each:
                    nc.sync.dma_start(
                        out=outT[m2 * P:(m2 + 1) * P, c * F:(c + 1) * F],
                        in_=os2[:, cc, :],
                    )
                elif cc == 1:
                    nc.sync.dma_start(
                        out=outT[m2 * P:(m2 + 1) * P, su * ISUP:(su + 1) * ISUP],
                        in_=os2[:].rearrange("p c f -> p (c f)"),
                    )

            # ---- per-pair interleaved extras ----
            def K_(po, c):
                return lambda: qk_chunk(wk_sb, bk_col, KT, xk_sb, po, c)

            def Q_(po, c):
                return lambda: qk_chunk(wq_sb, bq_col, QT, xq_sb, po, c)

            def OP_(m2, su, cc):
                return lambda: op_group(m2, su, cc)

            extras_by_pair = {
                0: {1: [K_(0, 1)], 2: [K_(0, 2)], 3: [K_(0, 3)]},
                1: {9: [K_(1, 0)], 10: [K_(1, 1)], 11: [K_(1, 2)],
                    12: [K_(1, 3)], 13: [Q_(1, 0)], 14: [Q_(1, 1)]},
                2: {8: [Q_(1, 2)], 10: [Q_(1, 3)], 12: [Q_(0, 2)],
                    14: [Q_(0, 3)]},
                5: {3: [OP_(0, 0, 0)], 5: [OP_(0, 0, 1)], 7: [OP_(1, 0, 0)],
                    9: [OP_(1, 0, 1)]},
                6: {1: [OP_(2, 0, 0)], 3: [OP_(2, 0, 1)], 5: [OP_(3, 0, 0)],
                    7: [OP_(3, 0, 1)]},
            }

            # ---- attention pairs ----
            # PV for pair p: 8 ib-major runs (16 consecutive matmuls each)
            # interleaved into pair p+1's jt loop. Last pair: jt-halves.
            pairs = [(su, h) for su in range(NSUP) for h in range(NH)]
            LAST = len(pairs) - 1
            LOJT = 10
            LO_SCHED = [[0, 1], [2, 3], [4], [5], [6], [7]]
            state = {}

            def pv_run(p, ib, jt0=0, njt=JT, tgt_pair=None, fresh=True):
                ps = state[p]
                if tgt_pair is None:
                    if ib == 0 and fresh:
                        ps["xpa"] = ps_xp.tile([P, 4 * D1], f32, tag="xpa",
                                               name=f"xpa_{p}")
                        ps["xpb"] = ps_xp.tile([P, 4 * D1], f32, tag="xpb",
                                               name=f"xpb_{p}")
                    tgt = ps["xpa"] if ib < 4 else ps["xpb"]
                else:
                    tgt = tgt_pair[0] if ib < 4 else tgt_pair[1]
                ib2 = ib % 4
                hp = ps["h"]
                for jj in range(njt):
                    jt = jt0 + jj
                    nc.tensor.matmul(
                        tgt[:, ib2 * D1:(ib2 + 1) * D1],
                        ps["et"][jt][:, ib * P:(ib + 1) * P],
                        V2[:, jt, hp, :],
                        start=(jj == 0),
                        stop=(jj == njt - 1),
                    )

            def drain_pair(p):
                ps = state[p]
                h, su = ps["h"], ps["su"]
                xn2 = emit_norm(p, h, su, ps["xpa"], ps["xpb"])
                if h % 2 == 1:
                    emit_tp(h // 2, su, xn2)
                    del xn2_hold[(h // 2, su)]
                del state[p]

            for pi, (su, h) in enumerate(pairs):
                qoff = (h % 2) * DK
                qpo = h // 2
                isl = su * ISUP
                extras = extras_by_pair.get(pi, {})
                state[pi] = {"h": h, "su": su, "et": []}

                def scores(jt):
                    st = ps_st.tile([P, ISUP], f32, tag="st",
                                    name=f"st_{pi}_{jt}")
                    for c2 in range(ISUP // F):
                        nc.tensor.matmul(
                            st[:, c2 * F:(c2 + 1) * F],
                            KT[qoff:qoff + DK, qpo, jt * P:(jt + 1) * P],
                            QT[qoff:qoff + DK, qpo,
                               isl + c2 * F:isl + (c2 + 1) * F],
                            start=True,
                            stop=True,
                        )
                    return st

                if pi == 0:
                    # first scores tile: c0 half only, so the exp stream can
                    # start as soon as QT c0 lands; c1 finishes inside jt0
                    st_prev = ps_st.tile([P, ISUP], f32, tag="st",
                                         name="st_0_0")
                    nc.tensor.matmul(
                        st_prev[:, 0:F],
                        KT[qoff:qoff + DK, qpo, 0:P],
                        QT[qoff:qoff + DK, qpo, isl:isl + F],
                        start=True, stop=True,
                    )
                else:
                    st_prev = scores(0)
                for jt in range(JT):
                    et = epool.tile([P, ISUP], bf16, tag="et",
                                    name=f"et_{pi}_{jt}")
                    if pi == 0 and jt == 0:
                        nc.scalar.activation(et[:, 0:F], st_prev[:, 0:F],
                                             Exp, scale=0.125)
                        qk_chunk(wq_sb, bq_col, QT, xq_sb, 0, 1)
                        nc.tensor.matmul(
                            st_prev[:, F:ISUP],
                            KT[qoff:qoff + DK, qpo, 0:P],
                            QT[qoff:qoff + DK, qpo, isl + F:isl + ISUP],
                            start=True, stop=True,
                        )
                        nc.scalar.activation(et[:, F:ISUP], st_prev[:, F:ISUP],
                                             Exp, scale=0.125)
                    else:
                        nc.scalar.activation(et[:], st_prev[:], Exp,
                                             scale=0.125)
                    state[pi]["et"].append(et)
                    if jt + 1 < JT:
                        st_prev = scores(jt + 1)
                    if pi == 0:
                        v_chunk(jt)
                    if pi > 0 and jt < NIB:
                        pv_run(pi - 1, jt)
                    if pi > 0 and jt == NIB:
                        drain_pair(pi - 1)
                    if pi == LAST and jt >= 10:
                        # lo runs (jts 0-9) spread over jts 10-15
                        for ibx in LO_SCHED[jt - 10]:
                            pv_run(LAST, ibx, jt0=0, njt=LOJT,
                                   fresh=(ibx == 0))
                    for fn in extras.get(jt, []):
                        fn()

            # ---- tail: last pair hi-half + norm/tp/outproj per i-half ----
            ps7 = state[LAST]
            h7, su7 = ps7["h"], ps7["su"]
            # lo halves -> SBUF (overlaps the hi runs; DVE can read only one
            # PSUM operand per tensor_tensor)
            lo_sb = npool.tile([P, NIB * D1], f32, tag="losb")
            nc.vector.tensor_copy(lo_sb[:, 0:4 * D1], ps7["xpa"][:])
            nc.vector.tensor_copy(lo_sb[:, 4 * D1:NIB * D1], ps7["xpb"][:])
            hia = ps_pp.tile([P, 4 * D1], f32, tag="pp", name="hia")
            hib = ps_pp.tile([P, 4 * D1], f32, tag="pp", name="hib")
            for ib in range(NIB):
                pv_run(LAST, ib, jt0=LOJT, njt=JT - LOJT,
                       tgt_pair=(hia, hib))

            def z_of(xp):
                return xp[:].rearrange("p (ib c) -> p ib c", c=D1)[
                    :, :, DK:D1].rearrange("p ib one -> p (ib one)")

            # norm: z = (z_lo + eps) + z_hi fused; muls split DVE/ACT
            zs = npool.tile([P, NIB], f32, tag="zeps")
            nc.vector.scalar_tensor_tensor(
                out=zs[:, 0:4], in0=z_of(lo_sb)[:, 0:4], scalar=EPS,
                in1=z_of(hia), op0=mybir.AluOpType.add,
                op1=mybir.AluOpType.add)
            nc.vector.scalar_tensor_tensor(
                out=zs[:, 4:8], in0=z_of(lo_sb)[:, 4:8], scalar=EPS,
                in1=z_of(hib), op0=mybir.AluOpType.add,
                op1=mybir.AluOpType.add)
            rz7 = npool.tile([P, NIB], f32, tag="rz")
            nc.vector.reciprocal(rz7[:], zs[:])
            xn27 = get_xn2(h7 // 2, su7)
            xs7 = npool.tile([P, NIB, DK], f32, tag="xs7")

            def norm_half(ib_lo):
                for ib in range(ib_lo, ib_lo + 4):
                    hi = hia if ib < 4 else hib
                    ib2 = ib % 4
                    nc.vector.tensor_add(
                        xs7[:, ib, :], lo_sb[:, ib * D1:ib * D1 + DK],
                        hi[:, ib2 * D1:ib2 * D1 + DK],
                    )
                    if ib % 2 == 1:
                        nc.scalar.mul(xn27[:, ib, h7 % 2, :], xs7[:, ib, :],
                                      rz7[:, ib:ib + 1])
                    else:
                        nc.vector.tensor_scalar_mul(
                            xn27[:, ib, h7 % 2, :], xs7[:, ib, :],
                            rz7[:, ib:ib + 1])

            norm_half(0)
            norm_half(4)
            emit_tp(h7 // 2, su7, xn27, ib_lo=0, nib=4)
            for m2 in range(F // P):
                op_group(m2, 1, 0, eng_act=(m2 % 2 == 1), dma_each=True)
            emit_tp(h7 // 2, su7, xn27, ib_lo=4, nib=4)
            for m2 in range(F // P):
                op_group(m2, 1, 1, eng_act=(m2 % 2 == 1), dma_each=True)

    nc.compile()
    return nc


def _prep_in_maps(query, key, value, mask, Wq, bq, Wk, bk, Wv, bv, Wo,
                  mask_ones):
    ident = np.eye(P, dtype=np.float32).astype(BF)
    B = query.shape[0]
    xTs = {}
    for b in range(B):
        m01 = (mask[b, 0, :] != 0)
        xv_full = value[b] * m01[:, None].astype(np.float32)
        xTs[b] = (
            np.ascontiguousarray(key[b].T).astype(BF),
            np.ascontiguousarray(query[b].T).astype(BF),
            np.ascontiguousarray(xv_full.T).astype(BF),
            np.ascontiguousarray(
                m01.astype(np.float32).reshape(NT, P).T) if not mask_ones
            else None,
        )
    in_maps = []
    for c in range(8):
        b = c // 2
        hh = c % 2
        ob = slice(hh * OB, (hh + 1) * OB)
        xkT, xqT, xvT, mc = xTs[b]
        bqk_h = np.concatenate(
            [bk[ob].reshape(OB // P, P).T, bq[ob].reshape(OB // P, P).T],
            axis=1,
        )
        wkq_h = np.stack([
            np.ascontiguousarray(Wk[ob, :].T),
            np.ascontiguousarray(Wq[ob, :].T),
        ]).astype(BF)
        wv_h = np.ascontiguousarray(Wv[ob, :].T).astype(BF)
        m = {
            "xk": xkT,
            "xq": xqT,
            "xv": xvT,
            "wkq": wkq_h,
            "wvd": wv_h,
            "wo": np.ascontiguousarray(Wo[:, ob].T).astype(BF),
            "bqk": np.ascontiguousarray(bqk_h),
            "bvb": np.ascontiguousarray(np.tile(bv[ob][None, :], (P, 1))),
            "ident": ident,
        }
        if not mask_ones:
            m["mcol"] = mc
        in_maps.append(m)
    return in_maps


def kernel(query, key, value, mask, Wq, bq, Wk, bk, Wv, bv, Wo, bo):
    query = np.asarray(query, dtype=np.float32)
    key = np.asarray(key, dtype=np.float32)
    value = np.asarray(value, dtype=np.float32)
    mask = np.asarray(mask)
    Wq = np.asarray(Wq, dtype=np.float32)
    bq = np.asarray(bq, dtype=np.float32)
    Wk = np.asarray(Wk, dtype=np.float32)
    bk = np.asarray(bk, dtype=np.float32)
    Wv = np.asarray(Wv, dtype=np.float32)
    bv = np.asarray(bv, dtype=np.float32)
    Wo = np.asarray(Wo, dtype=np.float32)
    bo = np.asarray(bo, dtype=np.float32)

    mask_ones = bool(np.all(mask != 0))
    ckey = ("nc", mask_ones)
    if ckey not in _CACHE:
        _CACHE[ckey] = _build(mask_ones)
        _CACHE["nc"] = _CACHE[ckey]  # test.py reads _CACHE["nc"]
    nc = _CACHE[ckey]

    B = query.shape[0]
    in_maps = _prep_in_maps(
        query, key, value, mask, Wq, bq, Wk, bk, Wv, bv, Wo, mask_ones
    )
    res = run_bass_kernel_spmd(nc, in_maps, core_ids=list(range(8)))

    out = np.empty((B, T, F), dtype=np.float32)
    for b in range(B):
        acc = res.results[2 * b]["outT"] + res.results[2 * b + 1]["outT"]
        out[b] = acc.T + bo[None, :]
    return out


# revision 15
# speedup vs baseline: 1.5353x; 1.0074x over previous
"""Multi-head attention Trainium2 kernel (8 NeuronCores, SPMD), v4.

Problem: B=4, T=2048, n_feat=512, H=8 heads, d_k=64.
Sharding: core c -> batch b = c//2, head-half hh = c%2 (4 heads = 256 attn dims).

Design (ACT-exp-bound: 128 exps of [128,1024] = ~133us on the scalar engine is
the floor; everything else hides under it):
- Host pre-transposes + bf16-casts activations (x^T [512,2048]) and weights.
- Q^T/K^T projections in [o, t] layout, V in [t, o] layout with a ones column
  (softmax denominator falls out of the PV matmul).
- Scores S^T[j, i] per (head, i-super of 1024); exp on ACT, bf16 out.
- PV out[i-block(128), dk+1] with et stationary. TRN2 psum accumulation must
  be one contiguous run per region (interleaved/reopened groups silently drop
  a visit), so PV for pair p runs ib-major (16 consecutive matmuls per
  region) interleaved into pair p+1's jt loop; all 16 et tiles of a pair stay
  live in SBUF.
- Norm = reciprocal + per-partition scalar mul (denominator is psum col 64).
- Normalized x transposed on PE in 64x64 quadrants (tile_position) so both
  heads of a pair-half pack onto 128 partitions -> output projection
  contracts 2x128 instead of 4x64 (half the matmuls).
- Tail: last pair's PV split into jt-halves (lo into xpa/xpb, hi into pp-tag
  psum), norm/transpose/outproj pipelined per i-half, ACT helps after the exp
  stream ends. Head: packed DMA prefix + PE warmup to beat the pstate ramp.
Host sums the two head-half partials per batch, transposes, adds bo.
"""
import sys

sys.path.insert(0, "/opt/trn_rl_repo")

import numpy as np
import ml_dtypes

import concourse.bass as bass
import concourse.tile as tile
from concourse import bacc, mybir
from concourse.bass_utils import run_bass_kernel_spmd

P = 128
T = 2048
F = 512            # n_feat (projection contraction dim)
OB = 256           # per-core attention dims (4 heads x 64)
NH = 4             # local heads
DK = 64
NT = T // P        # 16 row tiles
FO = F // P        # 4 feature tiles
NSUP = 2           # i-supers per head
ISUP = T // NSUP   # 1024
NIB = ISUP // P    # 8 i-blocks per super
JT = NT            # 16 j tiles
EPS = 1e-8
D1 = DK + 1

f32 = mybir.dt.float32
bf16 = mybir.dt.bfloat16
BF = ml_dtypes.bfloat16
Exp = mybir.ActivationFunctionType.Exp

_CACHE = {}


def _build(mask_ones: bool):
    nc = bacc.Bacc("TRN2", target_bir_lowering=False, debug=False, num_devices=8)

    xk = nc.dram_tensor("xk", (F, T), bf16, kind="ExternalInput").ap()
    xq = nc.dram_tensor("xq", (F, T), bf16, kind="ExternalInput").ap()
    xv = nc.dram_tensor("xv", (F, T), bf16, kind="ExternalInput").ap()
    wkq = nc.dram_tensor("wkq", (2, F, OB), bf16, kind="ExternalInput").ap()
    wvd = nc.dram_tensor("wvd", (F, OB), bf16, kind="ExternalInput").ap()
    wo = nc.dram_tensor("wo", (OB, F), bf16, kind="ExternalInput").ap()
    bqk = nc.dram_tensor("bqk", (P, 4), f32, kind="ExternalInput").ap()
    bvb = nc.dram_tensor("bvb", (P, OB), f32, kind="ExternalInput").ap()
    ident = nc.dram_tensor("ident", (P, P), bf16, kind="ExternalInput").ap()
    if not mask_ones:
        mcol = nc.dram_tensor("mcol", (P, NT), f32, kind="ExternalInput").ap()
    outT = nc.dram_tensor("outT", (F, T), bf16, kind="ExternalOutput").ap()

    xk_r = xk.rearrange("(fo p) t -> p fo t", p=P)
    xq_r = xq.rearrange("(fo p) t -> p fo t", p=P)
    xv_r = xv.rearrange("(fo p) t -> p fo t", p=P)

    with tile.TileContext(nc) as tc:
        with tc.tile_pool(name="const", bufs=1) as cpool, \
             tc.tile_pool(name="act", bufs=1) as apool, \
             tc.tile_pool(name="persist", bufs=1) as ppool, \
             tc.tile_pool(name="et", bufs=26) as epool, \
             tc.tile_pool(name="norm", bufs=2) as npool, \
             tc.tile_pool(name="out", bufs=4) as opool, \
             tc.tile_pool(name="ps_st", bufs=2, space="PSUM") as ps_st, \
             tc.tile_pool(name="ps_pp", bufs=2, space="PSUM") as ps_pp, \
             tc.tile_pool(name="ps_xp", bufs=1, space="PSUM") as ps_xp:

            # ---- SBUF tiles ----
            wkqv_sb = cpool.tile([P, 3, FO, OB], bf16, tag="wkqv")
            wo_sb = cpool.tile([P, 2, F], bf16, tag="wo")
            bqk_sb = cpool.tile([P, 4], f32, tag="bqk")
            bv_sb = cpool.tile([P, OB], f32, tag="bv")
            id_sb = cpool.tile([P, P], bf16, tag="ident")
            ws = cpool.tile([P, OB], bf16, tag="warm")
            if not mask_ones:
                mc_sb = cpool.tile([P, NT], f32, tag="mcol")
                z3_sb = cpool.tile([P, NH, 1], f32, tag="z3")

            xk_sb = apool.tile([P, FO, T], bf16, tag="xk")
            xq_sb = apool.tile([P, FO, T], bf16, tag="xq")
            xv_sb = apool.tile([P, FO, T], bf16, tag="xv")

            KT = ppool.tile([P, OB // P, T], bf16, tag="KT")
            QT = ppool.tile([P, OB // P, T], bf16, tag="QT")
            V2 = ppool.tile([P, NT, NH, D1], bf16, tag="V2")
            xT2 = ppool.tile([P, 2, T], bf16, tag="xT2")

            wk_sb = wkqv_sb[:, 0, :, :]
            wq_sb = wkqv_sb[:, 1, :, :]
            wv_sb = wkqv_sb[:, 2, :, :]
            bk_col = bqk_sb[:, 0:2]
            bq_col = bqk_sb[:, 2:4]

            # ---- PE warmup: keep PE busy through the pstate ramp ----
            nc.vector.memset(ws[:], 0.01)
            for wi in range(26):
                wp = ps_st.tile([P, OB], f32, tag="st", name=f"warm_{wi}")
                nc.tensor.matmul(
                    wp[:, :OB], ws[:, 0:P], ws[:, 0:OB], start=True, stop=True
                )

            # ---- DMA plan (single SP queue, prioritized order) ----
            def dma_x(xr, dst, c):
                nc.sync.dma_start(
                    out=dst[:, :, c * F:(c + 1) * F], in_=xr[:, :, c * F:(c + 1) * F]
                )

            nc.scalar.dma_start(out=bqk_sb[:], in_=bqk[:])
            nc.scalar.dma_start(out=bv_sb[:], in_=bvb[:])
            nc.scalar.dma_start(out=id_sb[:], in_=ident[:])
            if not mask_ones:
                nc.scalar.dma_start(out=mc_sb[:], in_=mcol[:])
            nc.sync.dma_start(
                out=wkqv_sb[:, 0:2, :, :],
                in_=wkq.rearrange("w (fo p) o -> p w fo o", p=P),
            )
            dma_x(xk_r, xk_sb, 0)
            dma_x(xq_r, xq_sb, 0)
            dma_x(xq_r, xq_sb, 1)
            nc.sync.dma_start(
                out=wkqv_sb[:, 2, :, :],
                in_=wvd.rearrange("(fo p) o -> p fo o", p=P),
            )
            dma_x(xv_r, xv_sb, 0)
            dma_x(xk_r, xk_sb, 1)
            dma_x(xv_r, xv_sb, 1)
            dma_x(xk_r, xk_sb, 2)
            dma_x(xv_r, xv_sb, 2)
            dma_x(xk_r, xk_sb, 3)
            dma_x(xv_r, xv_sb, 3)
            dma_x(xq_r, xq_sb, 2)
            dma_x(xq_r, xq_sb, 3)
            nc.sync.dma_start(out=wo_sb[:], in_=wo.rearrange("(oh p) f -> p oh f", p=P))

            # V2 ones column (or mask column)
            nc.vector.memset(V2[:, :, :, DK:D1], 1.0)
            if not mask_ones:
                nc.vector.memset(z3_sb[:], 0.0)

            # ---- projection emitters ----
            def qk_chunk(w_sb, b_col, dst, x_sb, po, c):
                pp = ps_pp.tile([P, F], f32, tag="pp", name=f"qk_{po}_{c}")
                for fo in range(FO):
                    nc.tensor.matmul(
                        pp[:, :F],
                        w_sb[:, fo, po * P:(po + 1) * P],
                        x_sb[:, fo, c * F:(c + 1) * F],
                        start=(fo == 0),
                        stop=(fo == FO - 1),
                    )
                nc.vector.tensor_scalar_add(
                    dst[:, po, c * F:(c + 1) * F], pp[:, :F], b_col[:, po:po + 1]
                )

            def v_chunk(tb):
                pp = ps_pp.tile([P, F], f32, tag="pp", name=f"v_{tb}")
                for fo in range(FO):
                    nc.tensor.matmul(
                        pp[:, :OB],
                        xv_sb[:, fo, tb * P:(tb + 1) * P],
                        wv_sb[:, fo, :],
                        start=(fo == 0),
                        stop=(fo == FO - 1),
                    )
                if mask_ones:
                    nc.vector.tensor_add(
                        V2[:, tb, :, 0:DK],
                        pp[:, :OB].rearrange("p (h d) -> p h d", h=NH),
                        bv_sb[:].rearrange("p (h d) -> p h d", h=NH),
                    )
                else:
                    vt = npool.tile([P, NH, DK], f32, tag="vt")
                    nc.vector.tensor_add(
                        vt[:],
                        pp[:, :OB].rearrange("p (h d) -> p h d", h=NH),
                        bv_sb[:].rearrange("p (h d) -> p h d", h=NH),
                    )
                    nc.vector.tensor_scalar_mul(
                        V2[:, tb, :, 0:DK], vt[:], mc_sb[:, tb:tb + 1]
                    )
                    nc.vector.tensor_scalar_add(
                        V2[:, tb, :, DK:D1], z3_sb[:], mc_sb[:, tb:tb + 1]
                    )

            # ---- head start: K po0 c0, Q po0 c0-c1 ----
            qk_chunk(wk_sb, bk_col, KT, xk_sb, 0, 0)
            qk_chunk(wq_sb, bq_col, QT, xq_sb, 0, 0)

            # ---- norm / transpose / outproj emitters ----
            def emit_norm_mul(xn, ib, src_ap, rz, eng_act=False):
                if eng_act:
                    nc.scalar.mul(xn[:, ib, :], src_ap, rz[:, ib:ib + 1])
                else:
                    nc.vector.tensor_scalar_mul(xn[:, ib, :], src_ap,
                                                rz[:, ib:ib + 1])

            xn2_hold = {}

            def get_xn2(oh, su):
                if (oh, su) not in xn2_hold:
                    xn2_hold[(oh, su)] = npool.tile(
                        [P, NIB, 2, DK], bf16, tag="xn",
                        name=f"xn2_{oh}_{su}")
                return xn2_hold[(oh, su)]

            def emit_norm(pi, h, su, xpa, xpb):
                zeps = npool.tile([P, NIB], f32, tag="zeps")
                za = xpa[:].rearrange("p (ib c) -> p ib c", c=D1)[
                    :, :, DK:D1].rearrange("p ib one -> p (ib one)")
                zb = xpb[:].rearrange("p (ib c) -> p ib c", c=D1)[
                    :, :, DK:D1].rearrange("p ib one -> p (ib one)")
                nc.vector.tensor_scalar_add(zeps[:, 0:4], za, EPS)
                nc.vector.tensor_scalar_add(zeps[:, 4:8], zb, EPS)
                rz = npool.tile([P, NIB], f32, tag="rz")
                nc.vector.reciprocal(rz[:], zeps[:])
                xn2 = get_xn2(h // 2, su)
                for ib in range(NIB):
                    src = xpa if ib < 4 else xpb
                    ib2 = ib % 4
                    nc.vector.tensor_scalar_mul(
                        xn2[:, ib, h % 2, :],
                        src[:, ib2 * D1:ib2 * D1 + DK],
                        rz[:, ib:ib + 1],
                    )
                return xn2

            def emit_tp_ib(tp, xn2, ib):
                # full 128x128 transpose: [i, (h d)] -> [(h d), i]
                nc.tensor.transpose(
                    tp[:, ib * P:(ib + 1) * P],
                    xn2[:, ib, :, :].rearrange("p h d -> p (h d)"),
                    id_sb[:],
                )

            def emit_tp(oh, su, xn2, ib_lo=0, nib=NIB):
                tp = ps_pp.tile([P, ISUP], bf16, tag="pp",
                                name=f"tp_{oh}_{su}_{ib_lo}")
                for ib in range(ib_lo, ib_lo + nib):
                    emit_tp_ib(tp, xn2, ib)
                nc.vector.tensor_copy(
                    xT2[:, oh, su * ISUP + ib_lo * P:
                        su * ISUP + (ib_lo + nib) * P],
                    tp[:, ib_lo * P:(ib_lo + nib) * P],
                )

            os2_hold = {}

            def op_group(m2, su, cc, eng_act=False, dma_each=False):
                if cc == 0:
                    os2_hold[m2] = opool.tile([P, 2, F], bf16, tag="os2",
                                              name=f"os2_{su}_{m2}")
                os2 = os2_hold[m2]
                c = su * 2 + cc
                pp = ps_pp.tile([P, F], f32, tag="pp", name=f"op_{m2}_{c}")
                for oh in range(2):
                    nc.tensor.matmul(
                        pp[:, :F],
                        wo_sb[:, oh, m2 * P:(m2 + 1) * P],
                        xT2[:, oh, c * F:(c + 1) * F],
                        start=(oh == 0),
                        stop=(oh == 1),
                    )
                if eng_act:
                    nc.scalar.copy(os2[:, cc, :], pp[:, :F])
                else:
                    nc.vector.tensor_copy(os2[:, cc, :], pp[:, :F])
                if dma_each:
                    nc.sync.dma_start(
                        out=outT[m2 * P:(m2 + 1) * P, c * F:(c + 1) * F],
                        in_=os2[:, cc, :],
                    )
                elif cc == 1:
                    nc.sync.dma_start(
                        out=outT[m2 * P:(m2 + 1) * P, su * ISUP:(su + 1) * ISUP],
                        in_=os2[:].rearrange("p c f -> p (c f)"),
                    )

            # ---- per-pair interleaved extras ----
            def K_(po, c):
                return lambda: qk_chunk(wk_sb, bk_col, KT, xk_sb, po, c)

            def Q_(po, c):
                return lambda: qk_chunk(wq_sb, bq_col, QT, xq_sb, po, c)

            def OP_(m2, su, cc):
                return lambda: op_group(m2, su, cc)

            extras_by_pair = {
                0: {1: [K_(0, 1)], 2: [K_(0, 2)], 3: [K_(0, 3)]},
                1: {9: [K_(1, 0)], 10: [K_(1, 1)], 11: [K_(1, 2)],
                    12: [K_(1, 3)], 13: [Q_(1, 0)], 14: [Q_(1, 1)]},
                2: {8: [Q_(1, 2)], 10: [Q_(1, 3)], 12: [Q_(0, 2)],
                    14: [Q_(0, 3)]},
                5: {3: [OP_(0, 0, 0)], 5: [OP_(0, 0, 1)], 7: [OP_(1, 0, 0)],
                    9: [OP_(1, 0, 1)]},
                6: {1: [OP_(2, 0, 0)], 3: [OP_(2, 0, 1)], 5: [OP_(3, 0, 0)],
                    7: [OP_(3, 0, 1)]},
            }

            # ---- attention pairs ----
            # PV for pair p: 8 ib-major runs (16 consecutive matmuls each)
            # interleaved into pair p+1's jt loop. Last pair: jt-halves.
            pairs = [(su, h) for su in range(NSUP) for h in range(NH)]
            LAST = len(pairs) - 1
            LOJT = 10
            LO_SCHED = [[0, 1], [2, 3], [4], [5], [6], [7]]
            state = {}

            def pv_run(p, ib, jt0=0, njt=JT, tgt_pair=None, fresh=True):
                ps = state[p]
                if tgt_pair is None:
                    if ib == 0 and fresh:
                        ps["xpa"] = ps_xp.tile([P, 4 * D1], f32, tag="xpa",
                                               name=f"xpa_{p}")
                        ps["xpb"] = ps_xp.tile([P, 4 * D1], f32, tag="xpb",
                                               name=f"xpb_{p}")
                    tgt = ps["xpa"] if ib < 4 else ps["xpb"]
                else:
                    tgt = tgt_pair[0] if ib < 4 else tgt_pair[1]
                ib2 = ib % 4
                hp = ps["h"]
                for jj in range(njt):
                    jt = jt0 + jj
                    nc.tensor.matmul(
                        tgt[:, ib2 * D1:(ib2 + 1) * D1],
                        ps["et"][jt][:, ib * P:(ib + 1) * P],
                        V2[:, jt, hp, :],
                        start=(jj == 0),
                        stop=(jj == njt - 1),
                    )

            def drain_pair(p):
                ps = state[p]
                h, su = ps["h"], ps["su"]
                xn2 = emit_norm(p, h, su, ps["xpa"], ps["xpb"])
                if h % 2 == 1:
                    emit_tp(h // 2, su, xn2)
                    del xn2_hold[(h // 2, su)]
                del state[p]

            for pi, (su, h) in enumerate(pairs):
                qoff = (h % 2) * DK
                qpo = h // 2
                isl = su * ISUP
                extras = extras_by_pair.get(pi, {})
                state[pi] = {"h": h, "su": su, "et": []}

                def scores(jt):
                    st = ps_st.tile([P, ISUP], f32, tag="st",
                                    name=f"st_{pi}_{jt}")
                    for c2 in range(ISUP // F):
                        nc.tensor.matmul(
                            st[:, c2 * F:(c2 + 1) * F],
                            KT[qoff:qoff + DK, qpo, jt * P:(jt + 1) * P],
                            QT[qoff:qoff + DK, qpo,
                               isl + c2 * F:isl + (c2 + 1) * F],
                            start=True,
                            stop=True,
                        )
                    return st

                if pi == 0:
                    # first scores tile: c0 half only, so the exp stream can
                    # start as soon as QT c0 lands; c1 finishes inside jt0
                    st_prev = ps_st.tile([P, ISUP], f32, tag="st",
                                         name="st_0_0")
                    nc.tensor.matmul(
                        st_prev[:, 0:F],
                        KT[qoff:qoff + DK, qpo, 0:P],
                        QT[qoff:qoff + DK, qpo, isl:isl + F],
                        start=True, stop=True,
                    )
                else:
                    st_prev = scores(0)
                for jt in range(JT):
                    et = epool.tile([P, ISUP], bf16, tag="et",
                                    name=f"et_{pi}_{jt}")
                    if pi == 0 and jt == 0:
                        nc.scalar.activation(et[:, 0:F], st_prev[:, 0:F],
                                             Exp, scale=0.125)
                        qk_chunk(wq_sb, bq_col, QT, xq_sb, 0, 1)
                        nc.tensor.matmul(
                            st_prev[:, F:ISUP],
                            KT[qoff:qoff + DK, qpo, 0:P],
                            QT[qoff:qoff + DK, qpo, isl + F:isl + ISUP],
                            start=True, stop=True,
                        )
                        nc.scalar.activation(et[:, F:ISUP], st_prev[:, F:ISUP],
                                             Exp, scale=0.125)
                    else:
                        nc.scalar.activation(et[:], st_prev[:], Exp,
                                             scale=0.125)
                    state[pi]["et"].append(et)
                    if jt + 1 < JT:
                        st_prev = scores(jt + 1)
                    if pi == 0:
                        v_chunk(jt)
                    if pi > 0 and jt < NIB:
                        pv_run(pi - 1, jt)
                    if pi > 0 and jt == NIB:
                        drain_pair(pi - 1)
                    if pi == LAST and jt >= 10:
                        # lo runs (jts 0-9) spread over jts 10-15
                        for ibx in LO_SCHED[jt - 10]:
                            pv_run(LAST, ibx, jt0=0, njt=LOJT,
                                   fresh=(ibx == 0))
                    for fn in extras.get(jt, []):
                        fn()

            # ---- tail: last pair hi-half + norm/tp/outproj per i-half ----
            ps7 = state[LAST]
            h7, su7 = ps7["h"], ps7["su"]
            # lo halves -> SBUF (overlaps the hi runs; DVE can read only one
            # PSUM operand per tensor_tensor)
            lo_sb = npool.tile([P, NIB * D1], f32, tag="losb")
            nc.vector.tensor_copy(lo_sb[:, 0:4 * D1], ps7["xpa"][:])
            nc.vector.tensor_copy(lo_sb[:, 4 * D1:NIB * D1], ps7["xpb"][:])
            hia = ps_pp.tile([P, 4 * D1], f32, tag="pp", name="hia")
            hib = ps_pp.tile([P, 4 * D1], f32, tag="pp", name="hib")
            for ib in range(NIB):
                pv_run(LAST, ib, jt0=LOJT, njt=JT - LOJT,
                       tgt_pair=(hia, hib))

            def z_of(xp):
                return xp[:].rearrange("p (ib c) -> p ib c", c=D1)[
                    :, :, DK:D1].rearrange("p ib one -> p (ib one)")

            # norm: z = (z_lo + eps) + z_hi fused; muls split DVE/ACT
            zs = npool.tile([P, NIB], f32, tag="zeps")
            nc.vector.scalar_tensor_tensor(
                out=zs[:, 0:4], in0=z_of(lo_sb)[:, 0:4], scalar=EPS,
                in1=z_of(hia), op0=mybir.AluOpType.add,
                op1=mybir.AluOpType.add)
            nc.vector.scalar_tensor_tensor(
                out=zs[:, 4:8], in0=z_of(lo_sb)[:, 4:8], scalar=EPS,
                in1=z_of(hib), op0=mybir.AluOpType.add,
                op1=mybir.AluOpType.add)
            rz7 = npool.tile([P, NIB], f32, tag="rz")
            nc.vector.reciprocal(rz7[:], zs[:])
            xn27 = get_xn2(h7 // 2, su7)
            xs7 = npool.tile([P, NIB, DK], f32, tag="xs7")

            def norm_half(ib_lo):
                hi = hia if ib_lo < 4 else hib
                lo4 = lo_sb[:].rearrange("p (ib c) -> p ib c", c=D1)[
                    :, ib_lo:ib_lo + 4, 0:DK]
                hi4 = hi[:].rearrange("p (ib c) -> p ib c", c=D1)[
                    :, :, 0:DK]
                nc.vector.tensor_add(xs7[:, ib_lo:ib_lo + 4, :], lo4, hi4)
                for ib in range(ib_lo, ib_lo + 4):
                    if ib % 2 == 1:
                        nc.scalar.mul(xn27[:, ib, h7 % 2, :], xs7[:, ib, :],
                                      rz7[:, ib:ib + 1])
                    else:
                        nc.vector.tensor_scalar_mul(
                            xn27[:, ib, h7 % 2, :], xs7[:, ib, :],
                            rz7[:, ib:ib + 1])

            norm_half(0)
            norm_half(4)
            emit_tp(h7 // 2, su7, xn27, ib_lo=0, nib=4)
            for m2 in range(F // P):
                op_group(m2, 1, 0, eng_act=(m2 % 2 == 1), dma_each=True)
            emit_tp(h7 // 2, su7, xn27, ib_lo=4, nib=4)
            for m2 in range(F // P):
                op_group(m2, 1, 1, eng_act=(m2 % 2 == 1), dma_each=True)

    nc.compile()
    return nc


def _prep_in_maps(query, key, value, mask, Wq, bq, Wk, bk, Wv, bv, Wo,
                  mask_ones):
    ident = np.eye(P, dtype=np.float32).astype(BF)
    B = query.shape[0]
    xTs = {}
    for b in range(B):
        m01 = (mask[b, 0, :] != 0)
        xv_full = value[b] * m01[:, None].astype(np.float32)
        xTs[b] = (
            np.ascontiguousarray(key[b].T).astype(BF),
            np.ascontiguousarray(query[b].T).astype(BF),
            np.ascontiguousarray(xv_full.T).astype(BF),
            np.ascontiguousarray(
                m01.astype(np.float32).reshape(NT, P).T) if not mask_ones
            else None,
        )
    in_maps = []
    for c in range(8):
        b = c // 2
        hh = c % 2
        ob = slice(hh * OB, (hh + 1) * OB)
        xkT, xqT, xvT, mc = xTs[b]
        bqk_h = np.concatenate(
            [bk[ob].reshape(OB // P, P).T, bq[ob].reshape(OB // P, P).T],
            axis=1,
        )
        wkq_h = np.stack([
            np.ascontiguousarray(Wk[ob, :].T),
            np.ascontiguousarray(Wq[ob, :].T),
        ]).astype(BF)
        wv_h = np.ascontiguousarray(Wv[ob, :].T).astype(BF)
        m = {
            "xk": xkT,
            "xq": xqT,
            "xv": xvT,
            "wkq": wkq_h,
            "wvd": wv_h,
            "wo": np.ascontiguousarray(Wo[:, ob].T).astype(BF),
            "bqk": np.ascontiguousarray(bqk_h),
            "bvb": np.ascontiguousarray(np.tile(bv[ob][None, :], (P, 1))),
            "ident": ident,
        }
        if not mask_ones:
            m["mcol"] = mc
        in_maps.append(m)
    return in_maps


def kernel(query, key, value, mask, Wq, bq, Wk, bk, Wv, bv, Wo, bo):
    query = np.asarray(query, dtype=np.float32)
    key = np.asarray(key, dtype=np.float32)
    value = np.asarray(value, dtype=np.float32)
    mask = np.asarray(mask)
    Wq = np.asarray(Wq, dtype=np.float32)
    bq = np.asarray(bq, dtype=np.float32)
    Wk = np.asarray(Wk, dtype=np.float32)
    bk = np.asarray(bk, dtype=np.float32)
    Wv = np.asarray(Wv, dtype=np.float32)
    bv = np.asarray(bv, dtype=np.float32)
    Wo = np.asarray(Wo, dtype=np.float32)
    bo = np.asarray(bo, dtype=np.float32)

    mask_ones = bool(np.all(mask != 0))
    ckey = ("nc", mask_ones)
    if ckey not in _CACHE:
        _CACHE[ckey] = _build(mask_ones)
        _CACHE["nc"] = _CACHE[ckey]  # test.py reads _CACHE["nc"]
    nc = _CACHE[ckey]

    B = query.shape[0]
    in_maps = _prep_in_maps(
        query, key, value, mask, Wq, bq, Wk, bk, Wv, bv, Wo, mask_ones
    )
    res = run_bass_kernel_spmd(nc, in_maps, core_ids=list(range(8)))

    out = np.empty((B, T, F), dtype=np.float32)
    for b in range(B):
        acc = (np.asarray(res.results[2 * b]["outT"], dtype=np.float32)
               + np.asarray(res.results[2 * b + 1]["outT"], dtype=np.float32))
        out[b] = acc.T + bo[None, :]
    return out
